# revision 51
# baseline (speedup 1.0000x reference)
"""Trainium2 Bass kernel for nn_ClassBlock (dense_transformer, memory regime).

Strategy
--------
The ClassBlock only transforms x[:, 0, :] (the cls token); x[:, 1:, :] passes
through untouched (out[:, 1:, :] == x[:, 1:, :] bit-for-bit).  The device
kernel therefore computes ONLY the cls rows; the host splices the untouched
tail into the output buffer.  Shipping the 268 MB identity tail through the
NeuronCores would be pure dead HBM traffic.

Device-side sharding of the cls math ([16,1024] activations):
  * activations replicated on every core,
  * heavy MLP weights sharded: fc1 column-sharded, fc2 row-sharded (1/8 per
    core) with one 64 KB ReduceScatter,
  * each core emits its own 2 batch rows (one-hot select matmul on cls1 +
    its ReduceScatter shard of the MLP output + fc2_b/8 folded into each
    core's partial so the reduction itself applies the bias).

Latency-oriented v2 (178us -> target):
  * ONE activation table load: a manual InstLoadActFuncSet pins the combined
    exp+ln set; sigmoid/silu = x*recip(1+exp(-x)) with DVE reciprocal,
    gelu ~= x*sigmoid(1.702x), softplus = ln(1+exp(x)), LN rstd =
    exp(-0.5*ln(var+eps)).  (The compiler's greedy table picker otherwise
    reloads 1.28us tables on every sigmoid<->exp transition: 19 loads.)
  * LayerNorm gain/bias folded into the downstream matmul weights on the
    host wherever the LN output only feeds a matmul (y3->gm_proj,
    norm2->fc1); conv center-tap weight folded into in_proj columns; all
    small biases applied as K=1 ones-row matmuls accumulated in PSUM.
  * DMA queues: cls/ident/sel/bias-rows on the SP HWDGE ring (land ~3us),
    broadcast LN/elementwise vectors on the ACT ring, all bf16 weights on
    the gpsimd SWDGE ring; everything fits SBUF, no streaming.
  * L=1 structural simplifications (3x3 'SAME' depthwise conv on a 1x1 map
    == center tap; selective scan with L=1, h0=0 == u*(delta*B*C + D)).
"""

import numpy as np

B, NTOK, C = 16, 4097, 1024
NCORES = 8
BPC = B // NCORES            # batches per core
DG = C // 4                  # 256 per-group channels
DTRANK = 16
HID = 4 * C                  # 4096
RED = C // 16                # 64
FC1_SH = HID // NCORES       # 512 fc1 column shard
FC2_SH = HID // NCORES       # 512 fc2 row shard
EPS = 1e-5

# broadcast vecs rows (each row = 1024 f32, replicated over 16 partitions)
R_GMW, R_GMB, R_N1W, R_D, R_ONW, R_ONB = range(6)
NV = 6

# bias-row blob offsets (single partition, bf16, used as K=1 matmul rhs)
OFF_CB = 0            # 4 x 512: [conv_b(256) | zeros(256)] per group
OFF_SE1B = 2048       # 64
OFF_SE2B = 2112       # 1024
OFF_GMB = 3136        # 1024: gm_norm_b @ gm_proj_w + gm_proj_b
OFF_FC1B = 4160       # 512: norm2_b @ fc1[:, shard] + fc1_b[shard]
OFF_FC2B = 4672       # 1024: fc2_b / 8
NBROW = 6144

DEBUG_TAPS = False

_CACHE = {}
LAST_RESULT = None
TRACE = False


def _f32(a):
    return np.ascontiguousarray(np.asarray(a, dtype=np.float32))


def _build(debug_taps):
    import concourse.bass as bass
    import concourse.tile as tile
    from concourse import bacc, mybir

    f32 = mybir.dt.float32
    bf16 = mybir.dt.bfloat16
    AF = mybir.ActivationFunctionType

    # Bacc (not plain Bass): its compile() legalizes to <=1 sync wait per
    # instruction (generate_event_semaphores), which TRN2 codegen requires.
    nc = bacc.Bacc("TRN2", target_bir_lowering=False, num_devices=NCORES)

    # ---- I/O ------------------------------------------------------------
    cls_h = nc.dram_tensor("cls_all", [B, C], f32, kind="ExternalInput")
    clsb_h = nc.dram_tensor("clsb", [B, C], f32, kind="ExternalInput")
    id_h = nc.dram_tensor("ident16", [B, B], f32, kind="ExternalInput")
    smal_h = nc.dram_tensor("smal", [B, 6], f32, kind="ExternalInput")
    selb_h = nc.dram_tensor("selb", [B, 2], bf16, kind="ExternalInput")
    mod2_h = nc.dram_tensor("mod2b", [B, 2], bf16, kind="ExternalInput")
    brow_h = nc.dram_tensor("brow", [1, NBROW], bf16, kind="ExternalInput")
    vecs_h = nc.dram_tensor("vecs", [NV * 1024], bf16, kind="ExternalInput")
    se1w_h = nc.dram_tensor("se1w", [C, RED], bf16, kind="ExternalInput")
    se2w_h = nc.dram_tensor("se2w", [RED, C], bf16, kind="ExternalInput")
    ipw_h = nc.dram_tensor("ipw", [4, DG, 2 * DG], bf16, kind="ExternalInput")
    xpw_h = nc.dram_tensor("xpw", [4, DG, DTRANK + 2], bf16, kind="ExternalInput")
    dtwa_h = nc.dram_tensor("dtwa", [4 * DTRANK + 1, C], bf16, kind="ExternalInput")
    opw_h = nc.dram_tensor("opw", [4, DG, DG], bf16, kind="ExternalInput")
    gmw_h = nc.dram_tensor("gmw", [C, C], bf16, kind="ExternalInput")
    fc1_h = nc.dram_tensor("fc1s", [C, FC1_SH], bf16, kind="ExternalInput")
    fc2_h = nc.dram_tensor("fc2s", [FC2_SH, C], bf16, kind="ExternalInput")
    out_h = nc.dram_tensor("out", [BPC, C], f32, kind="ExternalOutput")
    dbg_h = None
    if debug_taps:
        dbg_h = nc.dram_tensor("dbg", [8, B, C], f32, kind="ExternalOutput")

    def bc16(ap):
        # broadcast a DRAM AP across 16 partitions (step-0 partition dim)
        return bass.AP(tensor=ap.tensor, offset=ap.offset, ap=[[0, B]] + ap.ap)

    from contextlib import ExitStack

    with tile.TileContext(nc) as tc, ExitStack() as ctx:
        singles = ctx.enter_context(tc.tile_pool(name="singles", bufs=1))
        a1k = ctx.enter_context(tc.tile_pool(name="a1k", bufs=3))
        tiny = ctx.enter_context(tc.tile_pool(name="tiny", bufs=2))
        tp = ctx.enter_context(tc.tile_pool(name="tp", bufs=1))
        stats = ctx.enter_context(tc.tile_pool(name="stats", bufs=4))
        ppt = ctx.enter_context(tc.tile_pool(name="ppt", bufs=2, space="PSUM"))
        pm5 = ctx.enter_context(tc.tile_pool(name="pm5", bufs=2, space="PSUM"))
        pm = ctx.enter_context(tc.tile_pool(name="pm", bufs=2, space="PSUM"))
        dram = ctx.enter_context(tc.tile_pool(name="dram", bufs=1, space="DRAM"))

        # pin the combined exp+ln activation table ONCE; every ACT func used
        # below (Exp/Ln/Relu/Identity/Copy) lives in this set, so the
        # compiler's table-load pass inserts nothing further.
        atl = mybir.InstLoadActFuncSet(
            name=nc.get_next_instruction_name(), ins=[], outs=[],
            act_func_set_id=6)
        atl.engine = mybir.EngineType.Activation
        nc.add_instruction(atl)

        # ---- small inputs on the SP ring (land first) -------------------
        cls_t = singles.tile([B, C], f32, tag="cls")
        nc.sync.dma_start(out=cls_t[:], in_=cls_h[:])
        ident = singles.tile([B, B], f32, tag="ident")
        nc.sync.dma_start(out=ident[:], in_=id_h[:])
        smal_t = singles.tile([B, 6], f32, tag="smal")
        nc.sync.dma_start(out=smal_t[:], in_=smal_h[:])
        selb_t = singles.tile([B, 2], bf16, tag="selb")
        nc.sync.dma_start(out=selb_t[:], in_=selb_h[:])
        mod2_t = singles.tile([B, 2], bf16, tag="mod2")
        nc.sync.dma_start(out=mod2_t[:], in_=mod2_h[:])
        brow = singles.tile([1, NBROW], bf16, tag="brow")
        nc.sync.dma_start(out=brow[:], in_=brow_h[:])

        # broadcast vecs + late-needed cls+norm1_b on the ACT ring.
        # (The manual table load above precedes these in the ACT queue, so
        # the first Ln doesn't wait behind two DMA descriptor generations.)
        vecs = singles.tile([B, NV * 1024], bf16, tag="vecs")
        nc.scalar.dma_start(out=vecs[:], in_=bc16(vecs_h[:]))
        clsb_t = singles.tile([B, C], f32, tag="clsb")
        nc.scalar.dma_start(out=clsb_t[:], in_=clsb_h[:])

        def vrow(row, n=1024, off=0):
            return vecs[:, row * 1024 + off: row * 1024 + off + n]

        def brw(off, n):
            return brow[:, off:off + n]

        # warm up the CC stream immediately (ungated, garbage data): the
        # first collective after the entry barrier pays a ~35-50us
        # spin-up/skew cost; paying it here overlaps it with the chain so
        # the real ReduceScatter below runs in ~10us.
        dwarm_in = dram.tile([1, 4], f32, tag="dwarm_in")
        dwarm_out = dram.tile([1, 4], f32, tag="dwarm_out")
        nc.gpsimd.collective_compute(
            "AllReduce", mybir.AluOpType.add,
            replica_groups=[list(range(NCORES))],
            ins=[dwarm_in[:].opt()], outs=[dwarm_out[:].opt()],
        )

        # ---- weights (gpsimd SWDGE ring), all resident ------------------
        se1w = singles.tile([128, 8, RED], bf16, tag="se1w")
        nc.gpsimd.dma_start(out=se1w[:], in_=se1w_h[:].rearrange("(t p) n -> p t n", p=128))
        ipw = singles.tile([128, 8, 512], bf16, tag="ipw")
        nc.gpsimd.dma_start(out=ipw[:], in_=ipw_h[:].rearrange("g (t p) n -> p (g t) n", p=128))
        se2w = singles.tile([RED, 2, 512], bf16, tag="se2w")
        nc.gpsimd.dma_start(out=se2w[:], in_=se2w_h[:].rearrange("k (c n) -> k c n", c=2))
        xpw = singles.tile([128, 8, DTRANK + 2], bf16, tag="xpw")
        nc.gpsimd.dma_start(out=xpw[:], in_=xpw_h[:].rearrange("g (t p) n -> p (g t) n", p=128))
        dtwa = singles.tile([4 * DTRANK + 1, C], bf16, tag="dtwa")
        nc.gpsimd.dma_start(out=dtwa[:], in_=dtwa_h[:])
        opw = singles.tile([128, 8, DG], bf16, tag="opw")
        nc.gpsimd.dma_start(out=opw[:], in_=opw_h[:].rearrange("g (t p) n -> p (g t) n", p=128))
        gmw = singles.tile([128, 8, C], bf16, tag="gmw")
        nc.gpsimd.dma_start(out=gmw[:], in_=gmw_h[:].rearrange("(t p) n -> p t n", p=128))
        fc1 = singles.tile([128, 8, FC1_SH], bf16, tag="fc1")
        nc.gpsimd.dma_start(out=fc1[:], in_=fc1_h[:].rearrange("(t p) n -> p t n", p=128))
        fc2 = singles.tile([128, 4, C], bf16, tag="fc2")
        nc.gpsimd.dma_start(out=fc2[:], in_=fc2_h[:].rearrange("(t p) n -> p t n", p=128))

        ones1 = singles.tile([1, B], bf16, tag="ones1")
        nc.vector.memset(ones1[:], 1.0)
        identb = singles.tile([B, B], bf16, tag="identb")
        nc.vector.tensor_copy(out=identb[:], in_=ident[:])

        # ---- helpers -----------------------------------------------------
        def ln_stats(x_sl, cdim):
            """bn stats + rstd; returns (nm, rstd) [B,1] f32 tiles."""
            nsub = max(1, cdim // 512)
            if nsub == 1:
                st = stats.tile([B, 6], f32, tag="st6")
                nc.vector.bn_stats(out=st[:], in_=x_sl)
            else:
                st = stats.tile([B, nsub, 6], f32, tag="st26")
                for s in range(nsub):
                    nc.vector.bn_stats(out=st[:, s, :], in_=x_sl[:, s * 512:(s + 1) * 512])
            mv = stats.tile([B, 2], f32, tag="mv")
            nc.vector.bn_aggr(out=mv[:], in_=st[:])
            # rstd = exp(-0.5*ln(var+eps))
            nc.scalar.activation(out=mv[:, 1:2], in_=mv[:, 1:2], func=AF.Ln,
                                 bias=smal_t[:, 3:4], scale=1.0)
            nc.scalar.activation(out=mv[:, 1:2], in_=mv[:, 1:2], func=AF.Exp,
                                 scale=-0.5)
            nm = stats.tile([B, 1], f32, tag="nm")
            nc.vector.scalar_tensor_tensor(
                out=nm[:], in0=mv[:, 0:1], scalar=-1.0, in1=mv[:, 1:2],
                op0=mybir.AluOpType.mult, op1=mybir.AluOpType.mult)
            return nm, mv

        def ln_apply(x_sl, out_sl, nm, mv):
            # (x - mean) * rstd as one ACT op: Identity(x*rstd + (-mean*rstd))
            nc.scalar.activation(out=out_sl, in_=x_sl, func=AF.Identity,
                                 bias=nm[:], scale=mv[:, 1:2])

        def transpose_in(x_sl, cdim, tag="tp", in_bf16=False):
            # [16, cdim] (sbuf) -> [128, cdim//128, 16] (sbuf, bf16).
            # All k-tiles land in ONE psum tile so a single wide copy
            # replaces kt narrow ones.
            kt = cdim // 128
            idn = identb if in_bf16 else ident
            pt = ppt.tile([128, kt, B], bf16 if in_bf16 else f32, tag="pt")
            for t in range(kt):
                nc.tensor.transpose(pt[:, t, :], x_sl[:, t * 128:(t + 1) * 128], idn[:])
            xT = tp.tile([128, kt, B], bf16, tag=tag)
            nc.vector.tensor_copy(out=xT[:], in_=pt[:])
            return xT

        def sigmoid_into(dst, src_sl, n, scale=1.0):
            """dst = sigmoid(scale*src) = exp(-ln(1+exp(-scale*src))).

            Pure 3-op ACT chain: the +1 rides Ln's bias operand (a ones
            column), and DVE reciprocal (2.9us/op) is avoided entirely;
            all funcs live in the pinned exp+ln table set."""
            hn = n // 2
            for h in range(2):
                sl = slice(h * hn, (h + 1) * hn)
                nc.scalar.activation(out=dst[:, sl], in_=src_sl[:, sl],
                                     func=AF.Exp, scale=-scale)
                nc.scalar.activation(out=dst[:, sl], in_=dst[:, sl], func=AF.Ln,
                                     bias=smal_t[:, 4:5], scale=1.0)
                nc.scalar.activation(out=dst[:, sl], in_=dst[:, sl],
                                     func=AF.Exp, scale=-1.0)

        def tap(i, src_sl, n=C):
            if dbg_h is not None:
                nc.scalar.dma_start(out=dbg_h[i, :, :n], in_=src_sl)

        ALU = mybir.AluOpType

        # ---- cls chain ---------------------------------------------------
        # xnr = LN-raw(cls); gm_norm gain/bias are folded into se1/in_proj
        # weights host-side, so the matmuls consume xnr directly.  The full
        # xn tensor (gain/bias applied) is only needed for the y2 multiply
        # much later; it is computed off the critical path below.
        xnr = singles.tile([B, C], bf16, tag="xnr")
        nm, mv = ln_stats(cls_t[:], C)
        ln_apply(cls_t[:], xnr[:], nm, mv)
        xnT = transpose_in(xnr[:], C, tag="xnT", in_bf16=True)

        # SE block: se = sigmoid(relu(xn@W1+b1)@W2+b2)
        seh_p = pm5.tile([B, RED], f32, tag="pm512")
        for t in range(8):
            nc.tensor.matmul(seh_p[:], lhsT=xnT[:, t, :], rhs=se1w[:, t, :],
                             start=(t == 0), stop=False)
        nc.tensor.matmul(seh_p[:], lhsT=ones1[:], rhs=brw(OFF_SE1B, RED),
                         start=False, stop=True)
        seh = tiny.tile([B, RED], f32, tag="seh")
        nc.scalar.activation(out=seh[:], in_=seh_p[:], func=AF.Relu)
        pt = ppt.tile([128, B], f32, tag="pt")
        nc.tensor.transpose(pt[:RED, :], seh[:], ident[:])
        sehT = tiny.tile([RED, B], bf16, tag="sehT")
        nc.vector.tensor_copy(out=sehT[:], in_=pt[:RED, :])
        se_p = pm.tile([B, C], f32, tag="pm1k")
        for n in range(2):
            nc.tensor.matmul(se_p[:, n * 512:(n + 1) * 512], lhsT=sehT[:],
                             rhs=se2w[:, n, :], start=True, stop=False)
            nc.tensor.matmul(se_p[:, n * 512:(n + 1) * 512], lhsT=ones1[:],
                             rhs=brw(OFF_SE2B + n * 512, 512), start=False, stop=True)
        se_t = singles.tile([B, C], bf16, tag="se")

        # in_proj (conv center-tap folded into xs columns; conv_b as K=1 row)
        u_pre = singles.tile([B, C], bf16, tag="upre")
        z_pre = singles.tile([B, C], bf16, tag="zpre")
        for g in range(4):
            xz_p = pm5.tile([B, 2 * DG], f32, tag="pm512")
            for t in range(2):
                gt = 2 * g + t
                nc.tensor.matmul(xz_p[:], lhsT=xnT[:, gt, :], rhs=ipw[:, gt, :],
                                 start=(t == 0), stop=False)
            nc.tensor.matmul(xz_p[:], lhsT=ones1[:], rhs=brw(OFF_CB + g * 512, 512),
                             start=False, stop=True)
            sl = slice(g * DG, (g + 1) * DG)
            nc.vector.tensor_copy(out=u_pre[:, sl], in_=xz_p[:, :DG])
            nc.vector.tensor_copy(out=z_pre[:, sl], in_=xz_p[:, DG:])

        # u = silu(u_pre)
        u_all = singles.tile([B, C], bf16, tag="uall")
        sigmoid_into(u_all, u_pre[:], C)
        nc.vector.tensor_mul(out=u_all[:], in0=u_all[:], in1=u_pre[:])
        uT = transpose_in(u_all[:], C, tag="uT", in_bf16=True)

        # off-critical-path work emitted here (PE is busy with x_dbl/dt):
        # the SE sigmoid and the full xn tensor for the y2 multiply
        sigmoid_into(se_t, se_p[:], C)
        tap(1, se_t[:])
        xn = singles.tile([B, C], bf16, tag="xn")
        nc.vector.tensor_mul(out=xn[:], in0=xnr[:], in1=vrow(R_GMW))
        nc.vector.tensor_add(out=xn[:], in0=xn[:], in1=vrow(R_GMB))
        tap(0, xn[:])

        # x_dbl: one [16,4,18] psum; dts gathered into [16,65] with ones col
        dtscat = singles.tile([B, 4 * DTRANK + 1], f32, tag="dtscat")
        nc.vector.memset(dtscat[:, 4 * DTRANK:], 1.0)
        xdb_p = pm5.tile([B, 4, DTRANK + 2], f32, tag="pm512")
        for g in range(4):
            for t in range(2):
                nc.tensor.matmul(xdb_p[:, g, :], lhsT=uT[:, 2 * g + t, :],
                                 rhs=xpw[:, 2 * g + t, :],
                                 start=(t == 0), stop=(t == 1))
        bcx = tiny.tile([B, 4, 2], f32, tag="bcx")
        nc.vector.tensor_copy(out=bcx[:], in_=xdb_p[:, :, DTRANK:DTRANK + 2])
        bc4 = tiny.tile([B, 4], f32, tag="bc4")
        nc.vector.tensor_mul(out=bc4[:], in0=bcx[:, :, 0:1].rearrange("b g o -> b (g o)"),
                             in1=bcx[:, :, 1:2].rearrange("b g o -> b (g o)"))
        for g in range(4):
            nc.vector.tensor_copy(out=dtscat[:, g * DTRANK:(g + 1) * DTRANK],
                                  in_=xdb_p[:, g, :DTRANK])
        ptd = ppt.tile([128, B], f32, tag="pt")
        nc.tensor.transpose(ptd[:4 * DTRANK + 1, :], dtscat[:], ident[:])
        dtsT = tiny.tile([4 * DTRANK + 1, B], bf16, tag="dtsT")
        nc.vector.tensor_copy(out=dtsT[:], in_=ptd[:4 * DTRANK + 1, :])

        # delta_in = dts@blockdiag(dtw) + dtb  (ones row); then
        # y = u * (softplus(delta_in) * B*C + D)
        dl_p = pm.tile([B, C], f32, tag="pm1k")
        for n in range(2):
            nc.tensor.matmul(dl_p[:, n * 512:(n + 1) * 512], lhsT=dtsT[:],
                             rhs=dtwa[:, n * 512:(n + 1) * 512], start=True, stop=True)
        y_t = singles.tile([B, C], bf16, tag="y")
        for h in range(2):
            sl = slice(h * 512, (h + 1) * 512)
            nc.scalar.activation(out=y_t[:, sl], in_=dl_p[:, sl], func=AF.Exp)
            nc.scalar.activation(out=y_t[:, sl], in_=y_t[:, sl], func=AF.Ln,
                                 bias=smal_t[:, 4:5], scale=1.0)
        for g in range(4):
            sl = slice(g * DG, (g + 1) * DG)
            nc.vector.scalar_tensor_tensor(
                out=y_t[:, sl], in0=y_t[:, sl], scalar=bc4[:, g:g + 1],
                in1=vrow(R_D, DG, g * DG), op0=ALU.mult, op1=ALU.add)
        nc.vector.tensor_mul(out=y_t[:], in0=y_t[:], in1=u_all[:])
        tap(2, y_t[:])

        # sz = silu(z_pre)  (emitted late: DVE/ACT free while PE does x_dbl)
        sz = singles.tile([B, C], bf16, tag="sz")
        sigmoid_into(sz, z_pre[:], C)
        nc.vector.tensor_mul(out=sz[:], in0=sz[:], in1=z_pre[:])

        # per-group out-norm LN (stats batched across the 4 groups), * silu(z)
        yn = a1k.tile([B, C], bf16, tag="a1kb")
        mv4 = stats.tile([B, 4, 2], f32, tag="mv4")
        for g in range(4):
            st_g = stats.tile([B, 6], f32, tag="st6")
            nc.vector.bn_stats(out=st_g[:], in_=y_t[:, g * DG:(g + 1) * DG])
            nc.vector.bn_aggr(out=mv4[:, g, :], in_=st_g[:])
        nc.scalar.activation(out=mv4[:, :, 1:2], in_=mv4[:, :, 1:2], func=AF.Ln,
                             bias=smal_t[:, 3:4], scale=1.0)
        nc.scalar.activation(out=mv4[:, :, 1:2], in_=mv4[:, :, 1:2], func=AF.Exp,
                             scale=-0.5)
        nm4 = stats.tile([B, 4], f32, tag="nm4")
        nc.vector.scalar_tensor_tensor(
            out=nm4[:], in0=mv4[:, :, 0:1].rearrange("b g o -> b (g o)"),
            scalar=-1.0, in1=mv4[:, :, 1:2].rearrange("b g o -> b (g o)"),
            op0=ALU.mult, op1=ALU.mult)
        for g in range(4):
            sl = slice(g * DG, (g + 1) * DG)
            nc.scalar.activation(out=yn[:, sl], in_=y_t[:, sl], func=AF.Identity,
                                 bias=nm4[:, g:g + 1], scale=mv4[:, g, 1:2])
        nc.vector.tensor_mul(out=yn[:], in0=yn[:], in1=vrow(R_ONW))
        nc.vector.tensor_add(out=yn[:], in0=yn[:], in1=vrow(R_ONB))
        nc.vector.tensor_mul(out=yn[:], in0=yn[:], in1=sz[:])

        # out_proj per group
        yzT = transpose_in(yn[:], C, tag="yzT", in_bf16=True)
        ycat = a1k.tile([B, C], bf16, tag="a1kb")
        for g in range(4):
            ys_p = pm5.tile([B, DG], f32, tag="pm512")
            for t in range(2):
                nc.tensor.matmul(ys_p[:], lhsT=yzT[:, 2 * g + t, :],
                                 rhs=opw[:, 2 * g + t, :],
                                 start=(t == 0), stop=(t == 1))
            nc.vector.tensor_copy(out=ycat[:, g * DG:(g + 1) * DG], in_=ys_p[:])

        # y2 = ycat * skip * xn * se;  y3 = LN-raw(y2)  (gain/bias folded
        # into gm weights host-side)
        nc.vector.scalar_tensor_tensor(
            out=ycat[:], in0=ycat[:], scalar=smal_t[:, 2:3], in1=xn[:],
            op0=ALU.mult, op1=ALU.mult)
        nc.vector.tensor_mul(out=ycat[:], in0=ycat[:], in1=se_t[:])
        y3 = a1k.tile([B, C], bf16, tag="a1kb")
        nm3, mv3 = ln_stats(ycat[:], C)
        ln_apply(ycat[:], y3[:], nm3, mv3)

        # a = y3raw @ gm'  (+ bias row)
        y3T = transpose_in(y3[:], C, tag="y3T", in_bf16=True)
        a_p = pm.tile([B, C], f32, tag="pm1k")
        for n in range(2):
            for t in range(8):
                nc.tensor.matmul(a_p[:, n * 512:(n + 1) * 512], lhsT=y3T[:, t, :],
                                 rhs=gmw[:, t, n * 512:(n + 1) * 512],
                                 start=(t == 0), stop=False)
            nc.tensor.matmul(a_p[:, n * 512:(n + 1) * 512], lhsT=ones1[:],
                             rhs=brw(OFF_GMB + n * 512, 512), start=False, stop=True)

        # cls1 = (cls + n1b) + LN(a)*n1w   (cls+norm1_b precomputed on host)
        aln = a1k.tile([B, C], bf16, tag="a1kb")
        nma, mva = ln_stats(a_p[:], C)
        ln_apply(a_p[:], aln[:], nma, mva)
        nc.vector.tensor_mul(out=aln[:], in0=aln[:], in1=vrow(R_N1W))
        cls1 = singles.tile([B, C], bf16, tag="cls1")
        nc.vector.tensor_add(out=cls1[:], in0=clsb_t[:], in1=aln[:])
        tap(4, cls1[:])

        # select rows of cls1 into the final psum now; the h2 rows
        # accumulate into the same banks after the ReduceScatter lands.
        fin_p = pm.tile([BPC, C], f32, tag="pm1k")
        for n in range(2):
            sl = slice(n * 512, (n + 1) * 512)
            nc.tensor.matmul(fin_p[:, sl], lhsT=selb_t[:], rhs=cls1[:, sl],
                             start=True, stop=False)

        # h = LN-raw(cls1)  (norm2 gain/bias folded into fc1 host-side)
        h_t = a1k.tile([B, C], bf16, tag="a1kb")
        nmh, mvh = ln_stats(cls1[:], C)
        ln_apply(cls1[:], h_t[:], nmh, mvh)
        hT = transpose_in(h_t[:], C, tag="hT", in_bf16=True)

        # fc1 shard + gelu(sigmoid approx)
        h1_p = pm5.tile([B, FC1_SH], f32, tag="pm512")
        for t in range(8):
            nc.tensor.matmul(h1_p[:], lhsT=hT[:, t, :], rhs=fc1[:, t, :],
                             start=(t == 0), stop=False)
        nc.tensor.matmul(h1_p[:], lhsT=ones1[:], rhs=brw(OFF_FC1B, FC1_SH),
                         start=False, stop=True)
        h1s = tiny.tile([B, FC1_SH], bf16, tag="h1s")
        sigmoid_into(h1s, h1_p[:], FC1_SH, scale=1.702)
        h1 = tiny.tile([B, FC1_SH], bf16, tag="h1")
        nc.vector.tensor_mul(out=h1[:], in0=h1s[:], in1=h1_p[:])
        tap(5, h1[:], FC1_SH)

        # fc2 shard partial (+ fc2_b/8 so the ReduceScatter applies the bias)
        h1T = transpose_in(h1[:], FC1_SH, tag="h1T", in_bf16=True)
        p_p = pm.tile([B, C], f32, tag="pm1k")
        for n in range(2):
            for t in range(4):
                nc.tensor.matmul(p_p[:, n * 512:(n + 1) * 512], lhsT=h1T[:, t, :],
                                 rhs=fc2[:, t, n * 512:(n + 1) * 512],
                                 start=(t == 0), stop=False)
            nc.tensor.matmul(p_p[:, n * 512:(n + 1) * 512], lhsT=ones1[:],
                             rhs=brw(OFF_FC2B + n * 512, 512), start=False, stop=True)
        p_s = a1k.tile([B, C], bf16, tag="a1kb")
        nc.scalar.copy(out=p_s[:, :512], in_=p_p[:, :512])
        nc.scalar.copy(out=p_s[:, 512:], in_=p_p[:, 512:])

        # One-round exchange instead of a 3-round ReduceScatter: AllToAll
        # hands each core the other cores' partials of ITS 2 batch rows;
        # the cross-core sum then rides the final PSUM accumulation as a
        # single [16->2] matmul with a row-parity selector (exact f32 adds,
        # and ~half the collective latency).
        cc_in = dram.tile([B, C], bf16, tag="cc_in")
        cc_out = dram.tile([B, C], bf16, tag="cc_out")
        nc.gpsimd.dma_start(out=cc_in[:], in_=p_s[:])
        nc.gpsimd.collective_compute(
            "AllToAll", mybir.AluOpType.bypass,
            replica_groups=[list(range(NCORES))],
            ins=[cc_in[:].opt()], outs=[cc_out[:].opt()],
        )
        a2a = tiny.tile([B, C], bf16, tag="a2a")
        nc.gpsimd.dma_start(out=a2a[:], in_=cc_out[:])

        # accumulate the summed MLP rows onto the pre-selected cls1 rows
        for n in range(2):
            sl = slice(n * 512, (n + 1) * 512)
            nc.tensor.matmul(fin_p[:, sl], lhsT=mod2_t[:], rhs=a2a[:, sl],
                             start=False, stop=True)
        orow = tiny.tile([BPC, C], f32, tag="orow")
        nc.scalar.copy(out=orow[:], in_=fin_p[:])
        nc.scalar.dma_start(out=out_h[:, :], in_=orow[:])

    nc.compile()
    return nc


def _prepare_in_maps(inputs):
    import ml_dtypes

    def _w(a):
        return np.ascontiguousarray(_f32(a).astype(ml_dtypes.bfloat16))

    x = np.asarray(inputs["x"])
    cls_all = _f32(x[:, 0, :])
    cw_center = _f32(inputs["ss_conv_w"])[:, :, 1, 1]        # [4, 256]
    conv_b = _f32(inputs["ss_conv_b"])                        # [4, 256]
    gmw_n = _f32(inputs["gm_norm_w"])
    gmb_n = _f32(inputs["gm_norm_b"])
    n2w = _f32(inputs["norm2_w"])
    n2b = _f32(inputs["norm2_b"])
    gm_proj_w = _f32(inputs["gm_proj_w"])
    dt_w = _f32(inputs["ss_dt_w"])                            # [4, 16, 256]
    dt_b = _f32(inputs["ss_dt_b"])                            # [4, 256]
    fc1_w = _f32(inputs["mlp_fc1_w"])
    fc1_b = _f32(inputs["mlp_fc1_b"])
    fc2_w = _f32(inputs["mlp_fc2_w"])
    fc2_b = _f32(inputs["mlp_fc2_b"])

    # conv center tap folded into the xs half of in_proj columns, then
    # gm_norm gain folded into the rows (the matmul consumes raw-LN xnr);
    # gm_norm bias lands in the conv-bias row.
    ipw_host = _f32(inputs["ss_in_proj"]).copy()              # [4, 256, 512]
    ip_bias = np.zeros((4, 2 * DG), np.float32)
    for g in range(4):
        ipw_host[g][:, :DG] *= cw_center[g][None, :]
        gsl = slice(g * DG, (g + 1) * DG)
        ip_bias[g] = gmb_n[gsl] @ ipw_host[g]
        ipw_host[g] *= gmw_n[gsl][:, None]

    # gm_norm folded into the SE first layer likewise
    se1w_host = _f32(inputs["se_fc1_w"]) * gmw_n[:, None]
    se1b_host = gmb_n @ _f32(inputs["se_fc1_w"]) + _f32(inputs["se_fc1_b"])

    # dt blockdiag + dtb ones-row
    dtwa = np.zeros((4 * DTRANK + 1, C), np.float32)
    for g in range(4):
        dtwa[g * DTRANK:(g + 1) * DTRANK, g * DG:(g + 1) * DG] = dt_w[g]
    dtwa[4 * DTRANK, :] = dt_b.reshape(-1)

    # y3-LN gain folded into gm_proj rows; bias -> row vector
    gmw_host = gm_proj_w * gmw_n[:, None]
    gm_bias = gmb_n @ gm_proj_w + _f32(inputs["gm_proj_b"])

    # norm2 gain folded into fc1 rows
    fc1_host = fc1_w * n2w[:, None]

    vecs = np.zeros((NV, 1024), np.float32)
    vecs[R_GMW] = gmw_n
    vecs[R_GMB] = gmb_n
    vecs[R_N1W] = _f32(inputs["norm1_w"])
    vecs[R_D] = _f32(inputs["ss_D"]).reshape(-1)
    vecs[R_ONW] = _f32(inputs["ss_out_norm_w"]).reshape(-1)
    vecs[R_ONB] = _f32(inputs["ss_out_norm_b"]).reshape(-1)

    brow_base = np.zeros((NBROW,), np.float32)
    for g in range(4):
        brow_base[OFF_CB + g * 512: OFF_CB + g * 512 + 2 * DG] = ip_bias[g]
        brow_base[OFF_CB + g * 512: OFF_CB + g * 512 + DG] += conv_b[g]
    brow_base[OFF_SE1B:OFF_SE1B + RED] = se1b_host
    brow_base[OFF_SE2B:OFF_SE2B + C] = _f32(inputs["se_fc2_b"])
    brow_base[OFF_GMB:OFF_GMB + C] = gm_bias
    brow_base[OFF_FC2B:OFF_FC2B + C] = fc2_b / NCORES

    skip = float(_f32(inputs["skip_scale"]).reshape(-1)[0])

    shared = {
        "cls_all": cls_all,
        "clsb": _f32(cls_all + _f32(inputs["norm1_b"])[None, :]),
        "ident16": np.eye(B, dtype=np.float32),
        "vecs": np.ascontiguousarray(_w(vecs).reshape(-1)),
        "se1w": _w(se1w_host),
        "se2w": _w(inputs["se_fc2_w"]),
        "ipw": _w(ipw_host),
        "xpw": _w(inputs["ss_x_proj"]),
        "dtwa": _w(dtwa),
        "opw": _w(inputs["ss_out_proj"]),
        "gmw": _w(gmw_host),
    }

    in_maps = []
    for i in range(NCORES):
        sh = slice(i * FC1_SH, (i + 1) * FC1_SH)
        brow = brow_base.copy()
        brow[OFF_FC1B:OFF_FC1B + FC1_SH] = n2b @ fc1_w[:, sh] + fc1_b[sh]
        smal = np.zeros((B, 6), np.float32)
        for j in range(BPC):
            smal[i * BPC + j, j] = 1.0
        smal[:, 2] = skip
        smal[:, 3] = EPS
        smal[:, 4] = 1.0
        m = dict(shared)
        m.update({
            "smal": smal,
            "selb": _w(smal[:, 0:2]),
            "mod2b": _w(np.eye(2, dtype=np.float32)[np.arange(B) % 2]),
            "brow": np.ascontiguousarray(_w(brow).reshape(1, NBROW)),
            "fc1s": _w(fc1_host[:, sh]),
            "fc2s": _w(fc2_w[i * FC2_SH:(i + 1) * FC2_SH, :]),
        })
        in_maps.append(m)
    return in_maps


def _install_trace_shims():
    """This image lacks ``antenv.axon_hooks`` and fish-bucket access; stub in
    the ctypes NTFF hook from trn_boot and make artifact upload a no-op."""
    import sys
    import types

    import concourse.bass_utils as bu

    bu.upload_artifacts = lambda tmpdir: f"local:{tmpdir}"
    if "antenv.axon_hooks" not in sys.modules:
        from trn_agent_boot.trn_boot import _ntff_profile_via_ctypes

        mod = types.ModuleType("antenv.axon_hooks")
        hook = _ntff_profile_via_ctypes("/opt/axon/libaxon_pjrt.so")
        mod.get_axon_ntff_profile_hook = lambda: hook
        mod.set_axon_ntff_profile_hook = lambda h: None
        sys.modules["antenv.axon_hooks"] = mod
        import antenv

        antenv.axon_hooks = mod


def kernel(**inputs):
    global LAST_RESULT
    from concourse.bass_utils import run_bass_kernel_spmd

    key = "dbg" if DEBUG_TAPS else "plain"
    if key not in _CACHE:
        _CACHE[key] = _build(DEBUG_TAPS)
    nc = _CACHE[key]

    kwargs = {}
    if TRACE:
        _install_trace_shims()
        tdir = "/root/problem/.trace_" + key
        import os
        import shutil

        shutil.rmtree(tdir, ignore_errors=True)
        os.makedirs(tdir, exist_ok=True)
        kwargs = {"tmpdir": tdir}

    in_maps = _prepare_in_maps(inputs)
    res = run_bass_kernel_spmd(nc, in_maps, list(range(NCORES)), trace=TRACE, **kwargs)
    LAST_RESULT = res
    # device computed only the cls rows; the tail is the identity
    out = np.array(inputs["x"], dtype=np.float32, copy=True)
    out[:, 0, :] = np.concatenate([res.results[i]["out"] for i in range(NCORES)], axis=0)
    return out


# revision 52
# speedup vs baseline: 1.0233x; 1.0233x over previous
"""Trainium2 Bass kernel for nn_ClassBlock (dense_transformer, memory regime).

Strategy
--------
The ClassBlock only transforms x[:, 0, :] (the cls token); x[:, 1:, :] passes
through untouched (out[:, 1:, :] == x[:, 1:, :] bit-for-bit).  The device
kernel therefore computes ONLY the cls rows; the host splices the untouched
tail into the output buffer.  Shipping the 268 MB identity tail through the
NeuronCores would be pure dead HBM traffic.

Device-side sharding of the cls math ([16,1024] activations):
  * activations replicated on every core,
  * heavy MLP weights sharded: fc1 column-sharded, fc2 row-sharded (1/8 per
    core) with one 64 KB ReduceScatter,
  * each core emits its own 2 batch rows (one-hot select matmul on cls1 +
    its ReduceScatter shard of the MLP output + fc2_b/8 folded into each
    core's partial so the reduction itself applies the bias).

Latency-oriented v2 (178us -> target):
  * ONE activation table load: a manual InstLoadActFuncSet pins the combined
    exp+ln set; sigmoid/silu = x*recip(1+exp(-x)) with DVE reciprocal,
    gelu ~= x*sigmoid(1.702x), softplus = ln(1+exp(x)), LN rstd =
    exp(-0.5*ln(var+eps)).  (The compiler's greedy table picker otherwise
    reloads 1.28us tables on every sigmoid<->exp transition: 19 loads.)
  * LayerNorm gain/bias folded into the downstream matmul weights on the
    host wherever the LN output only feeds a matmul (y3->gm_proj,
    norm2->fc1); conv center-tap weight folded into in_proj columns; all
    small biases applied as K=1 ones-row matmuls accumulated in PSUM.
  * DMA queues: cls/ident/sel/bias-rows on the SP HWDGE ring (land ~3us),
    broadcast LN/elementwise vectors on the ACT ring, all bf16 weights on
    the gpsimd SWDGE ring; everything fits SBUF, no streaming.
  * L=1 structural simplifications (3x3 'SAME' depthwise conv on a 1x1 map
    == center tap; selective scan with L=1, h0=0 == u*(delta*B*C + D)).
"""

import numpy as np

B, NTOK, C = 16, 4097, 1024
NCORES = 8
BPC = B // NCORES            # batches per core
DG = C // 4                  # 256 per-group channels
DTRANK = 16
HID = 4 * C                  # 4096
RED = C // 16                # 64
FC1_SH = HID // NCORES       # 512 fc1 column shard
FC2_SH = HID // NCORES       # 512 fc2 row shard
EPS = 1e-5

# broadcast vecs rows (each row = 1024 f32, replicated over 16 partitions)
R_GMW, R_GMB, R_N1W, R_D, R_ONW, R_ONB = range(6)
NV = 6

# bias-row blob offsets (single partition, bf16, used as K=1 matmul rhs)
OFF_CB = 0            # 4 x 512: [conv_b(256) | zeros(256)] per group
OFF_SE1B = 2048       # 64
OFF_SE2B = 2112       # 1024
OFF_GMB = 3136        # 1024: gm_norm_b @ gm_proj_w + gm_proj_b
OFF_FC1B = 4160       # 512: norm2_b @ fc1[:, shard] + fc1_b[shard]
OFF_FC2B = 4672       # 1024: fc2_b / 8
NBROW = 6144

DEBUG_TAPS = False

_CACHE = {}
LAST_RESULT = None
TRACE = False


def _f32(a):
    return np.ascontiguousarray(np.asarray(a, dtype=np.float32))


def _build(debug_taps):
    import concourse.bass as bass
    import concourse.tile as tile
    from concourse import bacc, mybir

    f32 = mybir.dt.float32
    bf16 = mybir.dt.bfloat16
    AF = mybir.ActivationFunctionType

    # Bacc (not plain Bass): its compile() legalizes to <=1 sync wait per
    # instruction (generate_event_semaphores), which TRN2 codegen requires.
    nc = bacc.Bacc("TRN2", target_bir_lowering=False, num_devices=NCORES)

    # ---- I/O ------------------------------------------------------------
    cls_h = nc.dram_tensor("cls_all", [B, C], f32, kind="ExternalInput")
    clsb_h = nc.dram_tensor("clsb", [B, C], f32, kind="ExternalInput")
    id_h = nc.dram_tensor("ident16", [B, B], f32, kind="ExternalInput")
    smal_h = nc.dram_tensor("smal", [B, 6], f32, kind="ExternalInput")
    selb_h = nc.dram_tensor("selb", [B, 2], bf16, kind="ExternalInput")
    mod2_h = nc.dram_tensor("mod2b", [B, 2], bf16, kind="ExternalInput")
    brow_h = nc.dram_tensor("brow", [1, NBROW], bf16, kind="ExternalInput")
    vecs_h = nc.dram_tensor("vecs", [NV * 1024], bf16, kind="ExternalInput")
    se1w_h = nc.dram_tensor("se1w", [C, RED], bf16, kind="ExternalInput")
    se2w_h = nc.dram_tensor("se2w", [RED, C], bf16, kind="ExternalInput")
    ipw_h = nc.dram_tensor("ipw", [4, DG, 2 * DG], bf16, kind="ExternalInput")
    xpw_h = nc.dram_tensor("xpw", [4, DG, DTRANK + 2], bf16, kind="ExternalInput")
    dtwa_h = nc.dram_tensor("dtwa", [4 * DTRANK + 1, C], bf16, kind="ExternalInput")
    opw_h = nc.dram_tensor("opw", [4, DG, DG], bf16, kind="ExternalInput")
    gmw_h = nc.dram_tensor("gmw", [C, C], bf16, kind="ExternalInput")
    fc1_h = nc.dram_tensor("fc1s", [C, FC1_SH], bf16, kind="ExternalInput")
    fc2_h = nc.dram_tensor("fc2s", [FC2_SH, C], bf16, kind="ExternalInput")
    out_h = nc.dram_tensor("out", [BPC, C], f32, kind="ExternalOutput")
    dbg_h = None
    if debug_taps:
        dbg_h = nc.dram_tensor("dbg", [8, B, C], f32, kind="ExternalOutput")

    def bc16(ap):
        # broadcast a DRAM AP across 16 partitions (step-0 partition dim)
        return bass.AP(tensor=ap.tensor, offset=ap.offset, ap=[[0, B]] + ap.ap)

    from contextlib import ExitStack

    with tile.TileContext(nc) as tc, ExitStack() as ctx:
        singles = ctx.enter_context(tc.tile_pool(name="singles", bufs=1))
        a1k = ctx.enter_context(tc.tile_pool(name="a1k", bufs=3))
        tiny = ctx.enter_context(tc.tile_pool(name="tiny", bufs=2))
        tp = ctx.enter_context(tc.tile_pool(name="tp", bufs=1))
        stats = ctx.enter_context(tc.tile_pool(name="stats", bufs=4))
        ppt = ctx.enter_context(tc.tile_pool(name="ppt", bufs=2, space="PSUM"))
        pm5 = ctx.enter_context(tc.tile_pool(name="pm5", bufs=2, space="PSUM"))
        pm = ctx.enter_context(tc.tile_pool(name="pm", bufs=2, space="PSUM"))
        dram = ctx.enter_context(tc.tile_pool(name="dram", bufs=1, space="DRAM"))

        # pin the combined exp+ln activation table ONCE; every ACT func used
        # below (Exp/Ln/Relu/Identity/Copy) lives in this set, so the
        # compiler's table-load pass inserts nothing further.
        atl = mybir.InstLoadActFuncSet(
            name=nc.get_next_instruction_name(), ins=[], outs=[],
            act_func_set_id=6)
        atl.engine = mybir.EngineType.Activation
        nc.add_instruction(atl)

        # ---- small inputs on the SP ring (land first) -------------------
        cls_t = singles.tile([B, C], f32, tag="cls")
        nc.sync.dma_start(out=cls_t[:], in_=cls_h[:])
        ident = singles.tile([B, B], f32, tag="ident")
        nc.sync.dma_start(out=ident[:], in_=id_h[:])
        smal_t = singles.tile([B, 6], f32, tag="smal")
        nc.sync.dma_start(out=smal_t[:], in_=smal_h[:])
        selb_t = singles.tile([B, 2], bf16, tag="selb")
        nc.sync.dma_start(out=selb_t[:], in_=selb_h[:])
        mod2_t = singles.tile([B, 2], bf16, tag="mod2")
        nc.sync.dma_start(out=mod2_t[:], in_=mod2_h[:])
        brow = singles.tile([1, NBROW], bf16, tag="brow")
        nc.sync.dma_start(out=brow[:], in_=brow_h[:])

        # broadcast vecs + late-needed cls+norm1_b on the ACT ring.
        # (The manual table load above precedes these in the ACT queue, so
        # the first Ln doesn't wait behind two DMA descriptor generations.)
        vecs = singles.tile([B, NV * 1024], bf16, tag="vecs")
        nc.scalar.dma_start(out=vecs[:], in_=bc16(vecs_h[:]))
        clsb_t = singles.tile([B, C], f32, tag="clsb")
        nc.scalar.dma_start(out=clsb_t[:], in_=clsb_h[:])

        def vrow(row, n=1024, off=0):
            return vecs[:, row * 1024 + off: row * 1024 + off + n]

        def brw(off, n):
            return brow[:, off:off + n]

        # warm up the CC stream immediately (ungated, garbage data): the
        # first collective after the entry barrier pays a ~35-50us
        # spin-up/skew cost; paying it here overlaps it with the chain so
        # the real ReduceScatter below runs in ~10us.
        dwarm_in = dram.tile([1, 4], f32, tag="dwarm_in")
        dwarm_out = dram.tile([1, 4], f32, tag="dwarm_out")
        nc.gpsimd.collective_compute(
            "AllReduce", mybir.AluOpType.add,
            replica_groups=[list(range(NCORES))],
            ins=[dwarm_in[:].opt()], outs=[dwarm_out[:].opt()],
        )

        # ---- weights (gpsimd SWDGE ring), all resident ------------------
        se1w = singles.tile([128, 8, RED], bf16, tag="se1w")
        nc.gpsimd.dma_start(out=se1w[:], in_=se1w_h[:].rearrange("(t p) n -> p t n", p=128))
        ipw = singles.tile([128, 8, 512], bf16, tag="ipw")
        nc.gpsimd.dma_start(out=ipw[:], in_=ipw_h[:].rearrange("g (t p) n -> p (g t) n", p=128))
        se2w = singles.tile([RED, 2, 512], bf16, tag="se2w")
        nc.gpsimd.dma_start(out=se2w[:], in_=se2w_h[:].rearrange("k (c n) -> k c n", c=2))
        xpw = singles.tile([128, 8, DTRANK + 2], bf16, tag="xpw")
        nc.gpsimd.dma_start(out=xpw[:], in_=xpw_h[:].rearrange("g (t p) n -> p (g t) n", p=128))
        dtwa = singles.tile([4 * DTRANK + 1, C], bf16, tag="dtwa")
        nc.gpsimd.dma_start(out=dtwa[:], in_=dtwa_h[:])
        opw = singles.tile([128, 8, DG], bf16, tag="opw")
        nc.gpsimd.dma_start(out=opw[:], in_=opw_h[:].rearrange("g (t p) n -> p (g t) n", p=128))
        gmw = singles.tile([128, 8, C], bf16, tag="gmw")
        nc.gpsimd.dma_start(out=gmw[:], in_=gmw_h[:].rearrange("(t p) n -> p t n", p=128))
        fc1 = singles.tile([128, 8, FC1_SH], bf16, tag="fc1")
        nc.gpsimd.dma_start(out=fc1[:], in_=fc1_h[:].rearrange("(t p) n -> p t n", p=128))
        fc2 = singles.tile([128, 4, C], bf16, tag="fc2")
        nc.gpsimd.dma_start(out=fc2[:], in_=fc2_h[:].rearrange("(t p) n -> p t n", p=128))

        ones1 = singles.tile([1, B], bf16, tag="ones1")
        nc.vector.memset(ones1[:], 1.0)
        identb = singles.tile([B, B], bf16, tag="identb")
        nc.vector.tensor_copy(out=identb[:], in_=ident[:])

        # ---- helpers -----------------------------------------------------
        def ln_stats(x_sl, cdim):
            """bn stats + rstd; returns (nm, rstd) [B,1] f32 tiles."""
            nsub = max(1, cdim // 512)
            if nsub == 1:
                st = stats.tile([B, 6], f32, tag="st6")
                nc.vector.bn_stats(out=st[:], in_=x_sl)
            else:
                st = stats.tile([B, nsub, 6], f32, tag="st26")
                for s in range(nsub):
                    nc.vector.bn_stats(out=st[:, s, :], in_=x_sl[:, s * 512:(s + 1) * 512])
            mv = stats.tile([B, 2], f32, tag="mv")
            nc.vector.bn_aggr(out=mv[:], in_=st[:])
            # rstd = exp(-0.5*ln(var+eps))
            nc.scalar.activation(out=mv[:, 1:2], in_=mv[:, 1:2], func=AF.Ln,
                                 bias=smal_t[:, 3:4], scale=1.0)
            nc.scalar.activation(out=mv[:, 1:2], in_=mv[:, 1:2], func=AF.Exp,
                                 scale=-0.5)
            nm = stats.tile([B, 1], f32, tag="nm")
            nc.vector.scalar_tensor_tensor(
                out=nm[:], in0=mv[:, 0:1], scalar=-1.0, in1=mv[:, 1:2],
                op0=mybir.AluOpType.mult, op1=mybir.AluOpType.mult)
            return nm, mv

        def ln_apply(x_sl, out_sl, nm, mv):
            # (x - mean) * rstd as one ACT op: Identity(x*rstd + (-mean*rstd))
            nc.scalar.activation(out=out_sl, in_=x_sl, func=AF.Identity,
                                 bias=nm[:], scale=mv[:, 1:2])

        def transpose_in(x_sl, cdim, tag="tp", in_bf16=False):
            # [16, cdim] (sbuf) -> [128, cdim//128, 16] (sbuf, bf16).
            # All k-tiles land in ONE psum tile so a single wide copy
            # replaces kt narrow ones.
            kt = cdim // 128
            idn = identb if in_bf16 else ident
            pt = ppt.tile([128, kt, B], bf16 if in_bf16 else f32, tag="pt")
            for t in range(kt):
                nc.tensor.transpose(pt[:, t, :], x_sl[:, t * 128:(t + 1) * 128], idn[:])
            xT = tp.tile([128, kt, B], bf16, tag=tag)
            nc.vector.tensor_copy(out=xT[:], in_=pt[:])
            return xT

        def sigmoid_into(dst, src_sl, n, scale=1.0):
            """dst = sigmoid(scale*src) = exp(-ln(1+exp(-scale*src))).

            Pure 3-op ACT chain: the +1 rides Ln's bias operand (a ones
            column), and DVE reciprocal (2.9us/op) is avoided entirely;
            all funcs live in the pinned exp+ln table set."""
            hn = n // 2
            for h in range(2):
                sl = slice(h * hn, (h + 1) * hn)
                nc.scalar.activation(out=dst[:, sl], in_=src_sl[:, sl],
                                     func=AF.Exp, scale=-scale)
                nc.scalar.activation(out=dst[:, sl], in_=dst[:, sl], func=AF.Ln,
                                     bias=smal_t[:, 4:5], scale=1.0)
                nc.scalar.activation(out=dst[:, sl], in_=dst[:, sl],
                                     func=AF.Exp, scale=-1.0)

        def tap(i, src_sl, n=C):
            if dbg_h is not None:
                nc.scalar.dma_start(out=dbg_h[i, :, :n], in_=src_sl)

        ALU = mybir.AluOpType

        # ---- cls chain ---------------------------------------------------
        # xnr = LN-raw(cls); gm_norm gain/bias are folded into se1/in_proj
        # weights host-side, so the matmuls consume xnr directly.  The full
        # xn tensor (gain/bias applied) is only needed for the y2 multiply
        # much later; it is computed off the critical path below.
        xnr = singles.tile([B, C], bf16, tag="xnr")
        nm, mv = ln_stats(cls_t[:], C)
        ln_apply(cls_t[:], xnr[:], nm, mv)
        xnT = transpose_in(xnr[:], C, tag="xnT", in_bf16=True)

        # SE block: se = sigmoid(relu(xn@W1+b1)@W2+b2)
        seh_p = pm5.tile([B, RED], f32, tag="pm512")
        for t in range(8):
            nc.tensor.matmul(seh_p[:], lhsT=xnT[:, t, :], rhs=se1w[:, t, :],
                             start=(t == 0), stop=False)
        nc.tensor.matmul(seh_p[:], lhsT=ones1[:], rhs=brw(OFF_SE1B, RED),
                         start=False, stop=True)
        seh = tiny.tile([B, RED], f32, tag="seh")
        nc.scalar.activation(out=seh[:], in_=seh_p[:], func=AF.Relu)
        pt = ppt.tile([128, B], f32, tag="pt")
        nc.tensor.transpose(pt[:RED, :], seh[:], ident[:])
        sehT = tiny.tile([RED, B], bf16, tag="sehT")
        nc.vector.tensor_copy(out=sehT[:], in_=pt[:RED, :])
        se_p = pm.tile([B, C], f32, tag="pm1k")
        for n in range(2):
            nc.tensor.matmul(se_p[:, n * 512:(n + 1) * 512], lhsT=sehT[:],
                             rhs=se2w[:, n, :], start=True, stop=False)
            nc.tensor.matmul(se_p[:, n * 512:(n + 1) * 512], lhsT=ones1[:],
                             rhs=brw(OFF_SE2B + n * 512, 512), start=False, stop=True)
        se_t = singles.tile([B, C], bf16, tag="se")

        # in_proj (conv center-tap folded into xs columns; conv_b as K=1 row)
        u_pre = singles.tile([B, C], bf16, tag="upre")
        z_pre = singles.tile([B, C], bf16, tag="zpre")
        for g in range(4):
            xz_p = pm5.tile([B, 2 * DG], f32, tag="pm512")
            for t in range(2):
                gt = 2 * g + t
                nc.tensor.matmul(xz_p[:], lhsT=xnT[:, gt, :], rhs=ipw[:, gt, :],
                                 start=(t == 0), stop=False)
            nc.tensor.matmul(xz_p[:], lhsT=ones1[:], rhs=brw(OFF_CB + g * 512, 512),
                             start=False, stop=True)
            sl = slice(g * DG, (g + 1) * DG)
            nc.vector.tensor_copy(out=u_pre[:, sl], in_=xz_p[:, :DG])
            nc.vector.tensor_copy(out=z_pre[:, sl], in_=xz_p[:, DG:])

        # u = silu(u_pre)
        u_all = singles.tile([B, C], bf16, tag="uall")
        sigmoid_into(u_all, u_pre[:], C)
        nc.vector.tensor_mul(out=u_all[:], in0=u_all[:], in1=u_pre[:])
        uT = transpose_in(u_all[:], C, tag="uT", in_bf16=True)

        # off-critical-path work emitted here (PE is busy with x_dbl/dt):
        # the SE sigmoid and the full xn tensor for the y2 multiply
        sigmoid_into(se_t, se_p[:], C)
        tap(1, se_t[:])
        xn = singles.tile([B, C], bf16, tag="xn")
        nc.vector.tensor_mul(out=xn[:], in0=xnr[:], in1=vrow(R_GMW))
        nc.vector.tensor_add(out=xn[:], in0=xn[:], in1=vrow(R_GMB))
        tap(0, xn[:])

        # x_dbl: one [16,4,18] psum; dts gathered into [16,65] with ones col
        dtscat = singles.tile([B, 4 * DTRANK + 1], f32, tag="dtscat")
        nc.vector.memset(dtscat[:, 4 * DTRANK:], 1.0)
        xdb_p = pm5.tile([B, 4, DTRANK + 2], f32, tag="pm512")
        for g in range(4):
            for t in range(2):
                nc.tensor.matmul(xdb_p[:, g, :], lhsT=uT[:, 2 * g + t, :],
                                 rhs=xpw[:, 2 * g + t, :],
                                 start=(t == 0), stop=(t == 1))
        bcx = tiny.tile([B, 4, 2], f32, tag="bcx")
        nc.vector.tensor_copy(out=bcx[:], in_=xdb_p[:, :, DTRANK:DTRANK + 2])
        bc4 = tiny.tile([B, 4], f32, tag="bc4")
        nc.vector.tensor_mul(out=bc4[:], in0=bcx[:, :, 0:1].rearrange("b g o -> b (g o)"),
                             in1=bcx[:, :, 1:2].rearrange("b g o -> b (g o)"))
        for g in range(4):
            nc.vector.tensor_copy(out=dtscat[:, g * DTRANK:(g + 1) * DTRANK],
                                  in_=xdb_p[:, g, :DTRANK])
        ptd = ppt.tile([128, B], f32, tag="pt")
        nc.tensor.transpose(ptd[:4 * DTRANK + 1, :], dtscat[:], ident[:])
        dtsT = tiny.tile([4 * DTRANK + 1, B], bf16, tag="dtsT")
        nc.vector.tensor_copy(out=dtsT[:], in_=ptd[:4 * DTRANK + 1, :])

        # delta_in = dts@blockdiag(dtw) + dtb  (ones row); then
        # y = u * (softplus(delta_in) * B*C + D)
        dl_p = pm.tile([B, C], f32, tag="pm1k")
        for n in range(2):
            nc.tensor.matmul(dl_p[:, n * 512:(n + 1) * 512], lhsT=dtsT[:],
                             rhs=dtwa[:, n * 512:(n + 1) * 512], start=True, stop=True)
        y_t = singles.tile([B, C], bf16, tag="y")
        for h in range(2):
            sl = slice(h * 512, (h + 1) * 512)
            nc.scalar.activation(out=y_t[:, sl], in_=dl_p[:, sl], func=AF.Exp)
            nc.scalar.activation(out=y_t[:, sl], in_=y_t[:, sl], func=AF.Ln,
                                 bias=smal_t[:, 4:5], scale=1.0)
        for g in range(4):
            sl = slice(g * DG, (g + 1) * DG)
            nc.vector.scalar_tensor_tensor(
                out=y_t[:, sl], in0=y_t[:, sl], scalar=bc4[:, g:g + 1],
                in1=vrow(R_D, DG, g * DG), op0=ALU.mult, op1=ALU.add)
        nc.vector.tensor_mul(out=y_t[:], in0=y_t[:], in1=u_all[:])
        tap(2, y_t[:])

        # sz = silu(z_pre)  (emitted late: DVE/ACT free while PE does x_dbl)
        sz = singles.tile([B, C], bf16, tag="sz")
        sigmoid_into(sz, z_pre[:], C)
        nc.vector.tensor_mul(out=sz[:], in0=sz[:], in1=z_pre[:])

        # per-group out-norm LN (stats batched across the 4 groups), * silu(z)
        yn = a1k.tile([B, C], bf16, tag="a1kb")
        mv4 = stats.tile([B, 4, 2], f32, tag="mv4")
        for g in range(4):
            st_g = stats.tile([B, 6], f32, tag="st6")
            nc.vector.bn_stats(out=st_g[:], in_=y_t[:, g * DG:(g + 1) * DG])
            nc.vector.bn_aggr(out=mv4[:, g, :], in_=st_g[:])
        nc.scalar.activation(out=mv4[:, :, 1:2], in_=mv4[:, :, 1:2], func=AF.Ln,
                             bias=smal_t[:, 3:4], scale=1.0)
        nc.scalar.activation(out=mv4[:, :, 1:2], in_=mv4[:, :, 1:2], func=AF.Exp,
                             scale=-0.5)
        nm4 = stats.tile([B, 4], f32, tag="nm4")
        nc.vector.scalar_tensor_tensor(
            out=nm4[:], in0=mv4[:, :, 0:1].rearrange("b g o -> b (g o)"),
            scalar=-1.0, in1=mv4[:, :, 1:2].rearrange("b g o -> b (g o)"),
            op0=ALU.mult, op1=ALU.mult)
        for g in range(4):
            sl = slice(g * DG, (g + 1) * DG)
            nc.scalar.activation(out=yn[:, sl], in_=y_t[:, sl], func=AF.Identity,
                                 bias=nm4[:, g:g + 1], scale=mv4[:, g, 1:2])
        nc.vector.tensor_mul(out=yn[:], in0=yn[:], in1=vrow(R_ONW))
        nc.vector.tensor_add(out=yn[:], in0=yn[:], in1=vrow(R_ONB))
        nc.vector.tensor_mul(out=yn[:], in0=yn[:], in1=sz[:])

        # out_proj per group
        yzT = transpose_in(yn[:], C, tag="yzT", in_bf16=True)
        ycat = a1k.tile([B, C], bf16, tag="a1kb")
        for g in range(4):
            ys_p = pm5.tile([B, DG], f32, tag="pm512")
            for t in range(2):
                nc.tensor.matmul(ys_p[:], lhsT=yzT[:, 2 * g + t, :],
                                 rhs=opw[:, 2 * g + t, :],
                                 start=(t == 0), stop=(t == 1))
            nc.vector.tensor_copy(out=ycat[:, g * DG:(g + 1) * DG], in_=ys_p[:])

        # y2 = ycat * skip * xn * se;  y3 = LN-raw(y2)  (gain/bias folded
        # into gm weights host-side)
        nc.vector.scalar_tensor_tensor(
            out=ycat[:], in0=ycat[:], scalar=smal_t[:, 2:3], in1=xn[:],
            op0=ALU.mult, op1=ALU.mult)
        nc.vector.tensor_mul(out=ycat[:], in0=ycat[:], in1=se_t[:])
        y3 = a1k.tile([B, C], bf16, tag="a1kb")
        nm3, mv3 = ln_stats(ycat[:], C)
        ln_apply(ycat[:], y3[:], nm3, mv3)

        # a = y3raw @ gm'  (+ bias row)
        y3T = transpose_in(y3[:], C, tag="y3T", in_bf16=True)
        a_p = pm.tile([B, C], f32, tag="pm1k")
        for n in range(2):
            for t in range(8):
                nc.tensor.matmul(a_p[:, n * 512:(n + 1) * 512], lhsT=y3T[:, t, :],
                                 rhs=gmw[:, t, n * 512:(n + 1) * 512],
                                 start=(t == 0), stop=False)
            nc.tensor.matmul(a_p[:, n * 512:(n + 1) * 512], lhsT=ones1[:],
                             rhs=brw(OFF_GMB + n * 512, 512), start=False, stop=True)

        # cls1 = (cls + n1b) + LN(a)*n1w   (cls+norm1_b precomputed on host)
        aln = a1k.tile([B, C], bf16, tag="a1kb")
        nma, mva = ln_stats(a_p[:], C)
        ln_apply(a_p[:], aln[:], nma, mva)
        nc.vector.tensor_mul(out=aln[:], in0=aln[:], in1=vrow(R_N1W))
        cls1 = singles.tile([B, C], bf16, tag="cls1")
        nc.vector.tensor_add(out=cls1[:], in0=clsb_t[:], in1=aln[:])
        tap(4, cls1[:])

        # select rows of cls1 into the final psum now; the h2 rows
        # accumulate into the same banks after the ReduceScatter lands.
        fin_p = pm.tile([BPC, C], f32, tag="pm1k")
        for n in range(2):
            sl = slice(n * 512, (n + 1) * 512)
            nc.tensor.matmul(fin_p[:, sl], lhsT=selb_t[:], rhs=cls1[:, sl],
                             start=True, stop=False)

        # h = LN-raw(cls1)  (norm2 gain/bias folded into fc1 host-side)
        h_t = a1k.tile([B, C], bf16, tag="a1kb")
        nmh, mvh = ln_stats(cls1[:], C)
        ln_apply(cls1[:], h_t[:], nmh, mvh)
        hT = transpose_in(h_t[:], C, tag="hT", in_bf16=True)

        # fc1 shard + gelu(sigmoid approx)
        h1_p = pm5.tile([B, FC1_SH], f32, tag="pm512")
        for t in range(8):
            nc.tensor.matmul(h1_p[:], lhsT=hT[:, t, :], rhs=fc1[:, t, :],
                             start=(t == 0), stop=False)
        nc.tensor.matmul(h1_p[:], lhsT=ones1[:], rhs=brw(OFF_FC1B, FC1_SH),
                         start=False, stop=True)
        h1s = tiny.tile([B, FC1_SH], bf16, tag="h1s")
        sigmoid_into(h1s, h1_p[:], FC1_SH, scale=1.702)
        h1 = tiny.tile([B, FC1_SH], bf16, tag="h1")
        nc.vector.tensor_mul(out=h1[:], in0=h1s[:], in1=h1_p[:])
        tap(5, h1[:], FC1_SH)

        # fc2 shard partial (+ fc2_b/8 so the ReduceScatter applies the bias)
        h1T = transpose_in(h1[:], FC1_SH, tag="h1T", in_bf16=True)
        p_p = pm.tile([B, C], f32, tag="pm1k")
        for n in range(2):
            for t in range(4):
                nc.tensor.matmul(p_p[:, n * 512:(n + 1) * 512], lhsT=h1T[:, t, :],
                                 rhs=fc2[:, t, n * 512:(n + 1) * 512],
                                 start=(t == 0), stop=False)
            nc.tensor.matmul(p_p[:, n * 512:(n + 1) * 512], lhsT=ones1[:],
                             rhs=brw(OFF_FC2B + n * 512, 512), start=False, stop=True)
        p_s = a1k.tile([B, C], bf16, tag="a1kb")
        nc.scalar.copy(out=p_s[:, :512], in_=p_p[:, :512])
        nc.scalar.copy(out=p_s[:, 512:], in_=p_p[:, 512:])

        cc_in = dram.tile([B, C], bf16, tag="cc_in")
        cc_out = dram.tile([BPC, C], bf16, tag="cc_out")
        nc.gpsimd.dma_start(out=cc_in[:], in_=p_s[:])
        nc.gpsimd.collective_compute(
            "ReduceScatter", mybir.AluOpType.add,
            replica_groups=[list(range(NCORES))],
            ins=[cc_in[:].opt()], outs=[cc_out[:].opt()],
        )
        h2 = tiny.tile([BPC, C], bf16, tag="h2r")
        nc.gpsimd.dma_start(out=h2[:], in_=cc_out[:])

        # accumulate the reduced MLP rows onto the pre-selected cls1 rows
        for n in range(2):
            sl = slice(n * 512, (n + 1) * 512)
            nc.tensor.matmul(fin_p[:, sl], lhsT=identb[:2, :2], rhs=h2[:, sl],
                             start=False, stop=True)
        orow = tiny.tile([BPC, C], f32, tag="orow")
        nc.scalar.copy(out=orow[:], in_=fin_p[:])
        nc.scalar.dma_start(out=out_h[:, :], in_=orow[:])

    nc.compile()
    return nc


def _prepare_in_maps(inputs):
    import ml_dtypes

    def _w(a):
        return np.ascontiguousarray(_f32(a).astype(ml_dtypes.bfloat16))

    x = np.asarray(inputs["x"])
    cls_all = _f32(x[:, 0, :])
    cw_center = _f32(inputs["ss_conv_w"])[:, :, 1, 1]        # [4, 256]
    conv_b = _f32(inputs["ss_conv_b"])                        # [4, 256]
    gmw_n = _f32(inputs["gm_norm_w"])
    gmb_n = _f32(inputs["gm_norm_b"])
    n2w = _f32(inputs["norm2_w"])
    n2b = _f32(inputs["norm2_b"])
    gm_proj_w = _f32(inputs["gm_proj_w"])
    dt_w = _f32(inputs["ss_dt_w"])                            # [4, 16, 256]
    dt_b = _f32(inputs["ss_dt_b"])                            # [4, 256]
    fc1_w = _f32(inputs["mlp_fc1_w"])
    fc1_b = _f32(inputs["mlp_fc1_b"])
    fc2_w = _f32(inputs["mlp_fc2_w"])
    fc2_b = _f32(inputs["mlp_fc2_b"])

    # conv center tap folded into the xs half of in_proj columns, then
    # gm_norm gain folded into the rows (the matmul consumes raw-LN xnr);
    # gm_norm bias lands in the conv-bias row.
    ipw_host = _f32(inputs["ss_in_proj"]).copy()              # [4, 256, 512]
    ip_bias = np.zeros((4, 2 * DG), np.float32)
    for g in range(4):
        ipw_host[g][:, :DG] *= cw_center[g][None, :]
        gsl = slice(g * DG, (g + 1) * DG)
        ip_bias[g] = gmb_n[gsl] @ ipw_host[g]
        ipw_host[g] *= gmw_n[gsl][:, None]

    # gm_norm folded into the SE first layer likewise
    se1w_host = _f32(inputs["se_fc1_w"]) * gmw_n[:, None]
    se1b_host = gmb_n @ _f32(inputs["se_fc1_w"]) + _f32(inputs["se_fc1_b"])

    # dt blockdiag + dtb ones-row
    dtwa = np.zeros((4 * DTRANK + 1, C), np.float32)
    for g in range(4):
        dtwa[g * DTRANK:(g + 1) * DTRANK, g * DG:(g + 1) * DG] = dt_w[g]
    dtwa[4 * DTRANK, :] = dt_b.reshape(-1)

    # y3-LN gain folded into gm_proj rows; bias -> row vector
    gmw_host = gm_proj_w * gmw_n[:, None]
    gm_bias = gmb_n @ gm_proj_w + _f32(inputs["gm_proj_b"])

    # norm2 gain folded into fc1 rows
    fc1_host = fc1_w * n2w[:, None]

    vecs = np.zeros((NV, 1024), np.float32)
    vecs[R_GMW] = gmw_n
    vecs[R_GMB] = gmb_n
    vecs[R_N1W] = _f32(inputs["norm1_w"])
    vecs[R_D] = _f32(inputs["ss_D"]).reshape(-1)
    vecs[R_ONW] = _f32(inputs["ss_out_norm_w"]).reshape(-1)
    vecs[R_ONB] = _f32(inputs["ss_out_norm_b"]).reshape(-1)

    brow_base = np.zeros((NBROW,), np.float32)
    for g in range(4):
        brow_base[OFF_CB + g * 512: OFF_CB + g * 512 + 2 * DG] = ip_bias[g]
        brow_base[OFF_CB + g * 512: OFF_CB + g * 512 + DG] += conv_b[g]
    brow_base[OFF_SE1B:OFF_SE1B + RED] = se1b_host
    brow_base[OFF_SE2B:OFF_SE2B + C] = _f32(inputs["se_fc2_b"])
    brow_base[OFF_GMB:OFF_GMB + C] = gm_bias
    brow_base[OFF_FC2B:OFF_FC2B + C] = fc2_b / NCORES

    skip = float(_f32(inputs["skip_scale"]).reshape(-1)[0])

    shared = {
        "cls_all": cls_all,
        "clsb": _f32(cls_all + _f32(inputs["norm1_b"])[None, :]),
        "ident16": np.eye(B, dtype=np.float32),
        "vecs": np.ascontiguousarray(_w(vecs).reshape(-1)),
        "se1w": _w(se1w_host),
        "se2w": _w(inputs["se_fc2_w"]),
        "ipw": _w(ipw_host),
        "xpw": _w(inputs["ss_x_proj"]),
        "dtwa": _w(dtwa),
        "opw": _w(inputs["ss_out_proj"]),
        "gmw": _w(gmw_host),
    }

    in_maps = []
    for i in range(NCORES):
        sh = slice(i * FC1_SH, (i + 1) * FC1_SH)
        brow = brow_base.copy()
        brow[OFF_FC1B:OFF_FC1B + FC1_SH] = n2b @ fc1_w[:, sh] + fc1_b[sh]
        smal = np.zeros((B, 6), np.float32)
        for j in range(BPC):
            smal[i * BPC + j, j] = 1.0
        smal[:, 2] = skip
        smal[:, 3] = EPS
        smal[:, 4] = 1.0
        m = dict(shared)
        m.update({
            "smal": smal,
            "selb": _w(smal[:, 0:2]),
            "mod2b": _w(np.eye(2, dtype=np.float32)[np.arange(B) % 2]),
            "brow": np.ascontiguousarray(_w(brow).reshape(1, NBROW)),
            "fc1s": _w(fc1_host[:, sh]),
            "fc2s": _w(fc2_w[i * FC2_SH:(i + 1) * FC2_SH, :]),
        })
        in_maps.append(m)
    return in_maps


def _install_trace_shims():
    """This image lacks ``antenv.axon_hooks`` and fish-bucket access; stub in
    the ctypes NTFF hook from trn_boot and make artifact upload a no-op."""
    import sys
    import types

    import concourse.bass_utils as bu

    bu.upload_artifacts = lambda tmpdir: f"local:{tmpdir}"
    if "antenv.axon_hooks" not in sys.modules:
        from trn_agent_boot.trn_boot import _ntff_profile_via_ctypes

        mod = types.ModuleType("antenv.axon_hooks")
        hook = _ntff_profile_via_ctypes("/opt/axon/libaxon_pjrt.so")
        mod.get_axon_ntff_profile_hook = lambda: hook
        mod.set_axon_ntff_profile_hook = lambda h: None
        sys.modules["antenv.axon_hooks"] = mod
        import antenv

        antenv.axon_hooks = mod


def kernel(**inputs):
    global LAST_RESULT
    from concourse.bass_utils import run_bass_kernel_spmd

    key = "dbg" if DEBUG_TAPS else "plain"
    if key not in _CACHE:
        _CACHE[key] = _build(DEBUG_TAPS)
    nc = _CACHE[key]

    kwargs = {}
    if TRACE:
        _install_trace_shims()
        tdir = "/root/problem/.trace_" + key
        import os
        import shutil

        shutil.rmtree(tdir, ignore_errors=True)
        os.makedirs(tdir, exist_ok=True)
        kwargs = {"tmpdir": tdir}

    in_maps = _prepare_in_maps(inputs)
    res = run_bass_kernel_spmd(nc, in_maps, list(range(NCORES)), trace=TRACE, **kwargs)
    LAST_RESULT = res
    # device computed only the cls rows; the tail is the identity
    out = np.array(inputs["x"], dtype=np.float32, copy=True)
    out[:, 0, :] = np.concatenate([res.results[i]["out"] for i in range(NCORES)], axis=0)
    return out


# revision 55
# speedup vs baseline: 1.2711x; 1.2422x over previous
"""Trainium2 Bass kernel for nn_ClassBlock (dense_transformer, memory regime).

Strategy
--------
The ClassBlock only transforms x[:, 0, :] (the cls token); x[:, 1:, :] passes
through untouched (out[:, 1:, :] == x[:, 1:, :] bit-for-bit).  The device
kernel therefore computes ONLY the cls rows; the host splices the untouched
tail into the output buffer.  Shipping the 268 MB identity tail through the
NeuronCores would be pure dead HBM traffic.

Device-side sharding of the cls math ([16,1024] activations):
  * activations replicated on every core,
  * heavy MLP weights sharded: fc1 column-sharded, fc2 row-sharded (1/8 per
    core) with one 64 KB ReduceScatter,
  * each core emits its own 2 batch rows (one-hot select matmul on cls1 +
    its ReduceScatter shard of the MLP output + fc2_b/8 folded into each
    core's partial so the reduction itself applies the bias).

Latency-oriented v2 (178us -> target):
  * ONE activation table load: a manual InstLoadActFuncSet pins the combined
    exp+ln set; sigmoid/silu = x*recip(1+exp(-x)) with DVE reciprocal,
    gelu ~= x*sigmoid(1.702x), softplus = ln(1+exp(x)), LN rstd =
    exp(-0.5*ln(var+eps)).  (The compiler's greedy table picker otherwise
    reloads 1.28us tables on every sigmoid<->exp transition: 19 loads.)
  * LayerNorm gain/bias folded into the downstream matmul weights on the
    host wherever the LN output only feeds a matmul (y3->gm_proj,
    norm2->fc1); conv center-tap weight folded into in_proj columns; all
    small biases applied as K=1 ones-row matmuls accumulated in PSUM.
  * DMA queues: cls/ident/sel/bias-rows on the SP HWDGE ring (land ~3us),
    broadcast LN/elementwise vectors on the ACT ring, all bf16 weights on
    the gpsimd SWDGE ring; everything fits SBUF, no streaming.
  * L=1 structural simplifications (3x3 'SAME' depthwise conv on a 1x1 map
    == center tap; selective scan with L=1, h0=0 == u*(delta*B*C + D)).
"""

import numpy as np

B, NTOK, C = 16, 4097, 1024
NCORES = 8
BPC = B // NCORES            # batches per core
DG = C // 4                  # 256 per-group channels
DTRANK = 16
HID = 4 * C                  # 4096
RED = C // 16                # 64
FC1_SH = HID // NCORES       # 512 fc1 column shard
FC2_SH = HID // NCORES       # 512 fc2 row shard
EPS = 1e-5

# broadcast vecs rows (each row = 1024 f32, replicated over 16 partitions)
R_GMW, R_GMB, R_N1W, R_D, R_ONW, R_ONB = range(6)
NV = 6

# bias-row blob offsets (single partition, bf16, used as K=1 matmul rhs)
OFF_CB = 0            # 4 x 512: [conv_b(256) | zeros(256)] per group
OFF_SE1B = 2048       # 64
OFF_SE2B = 2112       # 1024
OFF_GMB = 3136        # 1024: gm_norm_b @ gm_proj_w + gm_proj_b
OFF_FC1B = 4160       # 512: norm2_b @ fc1[:, shard] + fc1_b[shard]
OFF_FC2B = 4672       # 1024: fc2_b / 8
NBROW = 6144

DEBUG_TAPS = False

_CACHE = {}
LAST_RESULT = None
TRACE = False


def _f32(a):
    return np.ascontiguousarray(np.asarray(a, dtype=np.float32))


def _build(debug_taps):
    import concourse.bass as bass
    import concourse.tile as tile
    from concourse import bacc, mybir

    f32 = mybir.dt.float32
    bf16 = mybir.dt.bfloat16
    AF = mybir.ActivationFunctionType

    # Bacc (not plain Bass): its compile() legalizes to <=1 sync wait per
    # instruction (generate_event_semaphores), which TRN2 codegen requires.
    nc = bacc.Bacc("TRN2", target_bir_lowering=False, num_devices=NCORES)

    # ---- I/O ------------------------------------------------------------
    cls_h = nc.dram_tensor("cls_all", [B, C], f32, kind="ExternalInput")
    clsb_h = nc.dram_tensor("clsb", [B, C], f32, kind="ExternalInput")
    id_h = nc.dram_tensor("ident16", [B, B], f32, kind="ExternalInput")
    smal_h = nc.dram_tensor("smal", [B, 6], f32, kind="ExternalInput")
    selb_h = nc.dram_tensor("selb", [B, 2], bf16, kind="ExternalInput")
    mod2_h = nc.dram_tensor("mod2b", [B, 2], bf16, kind="ExternalInput")
    brow_h = nc.dram_tensor("brow", [1, NBROW], bf16, kind="ExternalInput")
    vecs_h = nc.dram_tensor("vecs", [NV * 1024], bf16, kind="ExternalInput")
    se1w_h = nc.dram_tensor("se1w", [C, RED], bf16, kind="ExternalInput")
    se2w_h = nc.dram_tensor("se2w", [RED, C], bf16, kind="ExternalInput")
    ipw_h = nc.dram_tensor("ipw", [4, DG, 2 * DG], bf16, kind="ExternalInput")
    xpw_h = nc.dram_tensor("xpw", [4, DG, DTRANK + 2], bf16, kind="ExternalInput")
    dtwa_h = nc.dram_tensor("dtwa", [4 * DTRANK + 1, C], bf16, kind="ExternalInput")
    opw_h = nc.dram_tensor("opw", [4, DG, DG], bf16, kind="ExternalInput")
    gmw_h = nc.dram_tensor("gmw", [C, C], bf16, kind="ExternalInput")
    fc1_h = nc.dram_tensor("fc1s", [C, FC1_SH], bf16, kind="ExternalInput")
    fc2_h = nc.dram_tensor("fc2s", [FC2_SH, C], bf16, kind="ExternalInput")
    out_h = nc.dram_tensor("out", [BPC, C], f32, kind="ExternalOutput")
    h2o_h = nc.dram_tensor("h2o", [BPC, C], bf16, kind="ExternalOutput")
    dbg_h = None
    if debug_taps:
        dbg_h = nc.dram_tensor("dbg", [8, B, C], f32, kind="ExternalOutput")

    def bc16(ap):
        # broadcast a DRAM AP across 16 partitions (step-0 partition dim)
        return bass.AP(tensor=ap.tensor, offset=ap.offset, ap=[[0, B]] + ap.ap)

    from contextlib import ExitStack

    with tile.TileContext(nc) as tc, ExitStack() as ctx:
        singles = ctx.enter_context(tc.tile_pool(name="singles", bufs=1))
        a1k = ctx.enter_context(tc.tile_pool(name="a1k", bufs=3))
        tiny = ctx.enter_context(tc.tile_pool(name="tiny", bufs=2))
        tp = ctx.enter_context(tc.tile_pool(name="tp", bufs=1))
        stats = ctx.enter_context(tc.tile_pool(name="stats", bufs=4))
        ppt = ctx.enter_context(tc.tile_pool(name="ppt", bufs=2, space="PSUM"))
        pm5 = ctx.enter_context(tc.tile_pool(name="pm5", bufs=2, space="PSUM"))
        pm = ctx.enter_context(tc.tile_pool(name="pm", bufs=2, space="PSUM"))
        dram = ctx.enter_context(tc.tile_pool(name="dram", bufs=1, space="DRAM"))

        # pin the combined exp+ln activation table ONCE; every ACT func used
        # below (Exp/Ln/Relu/Identity/Copy) lives in this set, so the
        # compiler's table-load pass inserts nothing further.
        atl = mybir.InstLoadActFuncSet(
            name=nc.get_next_instruction_name(), ins=[], outs=[],
            act_func_set_id=6)
        atl.engine = mybir.EngineType.Activation
        nc.add_instruction(atl)

        # ---- small inputs on the SP ring (land first) -------------------
        cls_t = singles.tile([B, C], f32, tag="cls")
        nc.sync.dma_start(out=cls_t[:], in_=cls_h[:])
        ident = singles.tile([B, B], f32, tag="ident")
        nc.sync.dma_start(out=ident[:], in_=id_h[:])
        smal_t = singles.tile([B, 6], f32, tag="smal")
        nc.sync.dma_start(out=smal_t[:], in_=smal_h[:])
        selb_t = singles.tile([B, 2], bf16, tag="selb")
        nc.sync.dma_start(out=selb_t[:], in_=selb_h[:])
        mod2_t = singles.tile([B, 2], bf16, tag="mod2")
        nc.sync.dma_start(out=mod2_t[:], in_=mod2_h[:])
        brow = singles.tile([1, NBROW], bf16, tag="brow")
        nc.sync.dma_start(out=brow[:], in_=brow_h[:])

        # broadcast vecs + late-needed cls+norm1_b on the ACT ring.
        # (The manual table load above precedes these in the ACT queue, so
        # the first Ln doesn't wait behind two DMA descriptor generations.)
        vecs = singles.tile([B, NV * 1024], bf16, tag="vecs")
        nc.scalar.dma_start(out=vecs[:], in_=bc16(vecs_h[:]))
        clsb_t = singles.tile([B, C], f32, tag="clsb")
        nc.scalar.dma_start(out=clsb_t[:], in_=clsb_h[:])

        def vrow(row, n=1024, off=0):
            return vecs[:, row * 1024 + off: row * 1024 + off + n]

        def brw(off, n):
            return brow[:, off:off + n]

        # warm up the CC stream immediately (ungated, garbage data): the
        # first collective after the entry barrier pays a ~35-50us
        # spin-up/skew cost; paying it here overlaps it with the chain so
        # the real ReduceScatter below runs in ~10us.
        dwarm_in = dram.tile([1, 4], f32, tag="dwarm_in")
        dwarm_out = dram.tile([1, 4], f32, tag="dwarm_out")
        nc.gpsimd.collective_compute(
            "AllReduce", mybir.AluOpType.add,
            replica_groups=[list(range(NCORES))],
            ins=[dwarm_in[:].opt()], outs=[dwarm_out[:].opt()],
        )

        # ---- weights (gpsimd SWDGE ring), all resident ------------------
        se1w = singles.tile([128, 8, RED], bf16, tag="se1w")
        nc.gpsimd.dma_start(out=se1w[:], in_=se1w_h[:].rearrange("(t p) n -> p t n", p=128))
        ipw = singles.tile([128, 8, 512], bf16, tag="ipw")
        nc.gpsimd.dma_start(out=ipw[:], in_=ipw_h[:].rearrange("g (t p) n -> p (g t) n", p=128))
        se2w = singles.tile([RED, 2, 512], bf16, tag="se2w")
        nc.gpsimd.dma_start(out=se2w[:], in_=se2w_h[:].rearrange("k (c n) -> k c n", c=2))
        xpw = singles.tile([128, 8, DTRANK + 2], bf16, tag="xpw")
        nc.gpsimd.dma_start(out=xpw[:], in_=xpw_h[:].rearrange("g (t p) n -> p (g t) n", p=128))
        dtwa = singles.tile([4 * DTRANK + 1, C], bf16, tag="dtwa")
        nc.gpsimd.dma_start(out=dtwa[:], in_=dtwa_h[:])
        opw = singles.tile([128, 8, DG], bf16, tag="opw")
        nc.gpsimd.dma_start(out=opw[:], in_=opw_h[:].rearrange("g (t p) n -> p (g t) n", p=128))
        gmw = singles.tile([128, 8, C], bf16, tag="gmw")
        nc.gpsimd.dma_start(out=gmw[:], in_=gmw_h[:].rearrange("(t p) n -> p t n", p=128))
        fc1 = singles.tile([128, 8, FC1_SH], bf16, tag="fc1")
        nc.gpsimd.dma_start(out=fc1[:], in_=fc1_h[:].rearrange("(t p) n -> p t n", p=128))
        fc2 = singles.tile([128, 4, C], bf16, tag="fc2")
        nc.gpsimd.dma_start(out=fc2[:], in_=fc2_h[:].rearrange("(t p) n -> p t n", p=128))

        ones1 = singles.tile([1, B], bf16, tag="ones1")
        nc.vector.memset(ones1[:], 1.0)
        identb = singles.tile([B, B], bf16, tag="identb")
        nc.vector.tensor_copy(out=identb[:], in_=ident[:])

        # ---- helpers -----------------------------------------------------
        def ln_stats(x_sl, cdim):
            """bn stats + rstd; returns (nm, rstd) [B,1] f32 tiles."""
            nsub = max(1, cdim // 512)
            if nsub == 1:
                st = stats.tile([B, 6], f32, tag="st6")
                nc.vector.bn_stats(out=st[:], in_=x_sl)
            else:
                st = stats.tile([B, nsub, 6], f32, tag="st26")
                for s in range(nsub):
                    nc.vector.bn_stats(out=st[:, s, :], in_=x_sl[:, s * 512:(s + 1) * 512])
            mv = stats.tile([B, 2], f32, tag="mv")
            nc.vector.bn_aggr(out=mv[:], in_=st[:])
            # rstd = exp(-0.5*ln(var+eps))
            nc.scalar.activation(out=mv[:, 1:2], in_=mv[:, 1:2], func=AF.Ln,
                                 bias=smal_t[:, 3:4], scale=1.0)
            nc.scalar.activation(out=mv[:, 1:2], in_=mv[:, 1:2], func=AF.Exp,
                                 scale=-0.5)
            nm = stats.tile([B, 1], f32, tag="nm")
            nc.vector.scalar_tensor_tensor(
                out=nm[:], in0=mv[:, 0:1], scalar=-1.0, in1=mv[:, 1:2],
                op0=mybir.AluOpType.mult, op1=mybir.AluOpType.mult)
            return nm, mv

        def ln_apply(x_sl, out_sl, nm, mv):
            # (x - mean) * rstd as one ACT op: Identity(x*rstd + (-mean*rstd))
            nc.scalar.activation(out=out_sl, in_=x_sl, func=AF.Identity,
                                 bias=nm[:], scale=mv[:, 1:2])

        def transpose_in(x_sl, cdim, tag="tp", in_bf16=False):
            # [16, cdim] (sbuf) -> [128, cdim//128, 16] (sbuf, bf16).
            # All k-tiles land in ONE psum tile so a single wide copy
            # replaces kt narrow ones.
            kt = cdim // 128
            idn = identb if in_bf16 else ident
            pt = ppt.tile([128, kt, B], bf16 if in_bf16 else f32, tag="pt")
            for t in range(kt):
                nc.tensor.transpose(pt[:, t, :], x_sl[:, t * 128:(t + 1) * 128], idn[:])
            xT = tp.tile([128, kt, B], bf16, tag=tag)
            nc.vector.tensor_copy(out=xT[:], in_=pt[:])
            return xT

        def sigmoid_into(dst, src_sl, n, scale=1.0):
            """dst = sigmoid(scale*src) = exp(-ln(1+exp(-scale*src))).

            Pure 3-op ACT chain: the +1 rides Ln's bias operand (a ones
            column), and DVE reciprocal (2.9us/op) is avoided entirely;
            all funcs live in the pinned exp+ln table set."""
            hn = n // 2
            for h in range(2):
                sl = slice(h * hn, (h + 1) * hn)
                nc.scalar.activation(out=dst[:, sl], in_=src_sl[:, sl],
                                     func=AF.Exp, scale=-scale)
                nc.scalar.activation(out=dst[:, sl], in_=dst[:, sl], func=AF.Ln,
                                     bias=smal_t[:, 4:5], scale=1.0)
                nc.scalar.activation(out=dst[:, sl], in_=dst[:, sl],
                                     func=AF.Exp, scale=-1.0)

        def tap(i, src_sl, n=C):
            if dbg_h is not None:
                nc.scalar.dma_start(out=dbg_h[i, :, :n], in_=src_sl)

        ALU = mybir.AluOpType

        # ---- cls chain ---------------------------------------------------
        # xnr = LN-raw(cls); gm_norm gain/bias are folded into se1/in_proj
        # weights host-side, so the matmuls consume xnr directly.  The full
        # xn tensor (gain/bias applied) is only needed for the y2 multiply
        # much later; it is computed off the critical path below.
        xnr = singles.tile([B, C], bf16, tag="xnr")
        nm, mv = ln_stats(cls_t[:], C)
        ln_apply(cls_t[:], xnr[:], nm, mv)
        xnT = transpose_in(xnr[:], C, tag="xnT", in_bf16=True)

        # SE block: se = sigmoid(relu(xn@W1+b1)@W2+b2)
        seh_p = pm5.tile([B, RED], f32, tag="pm512")
        for t in range(8):
            nc.tensor.matmul(seh_p[:], lhsT=xnT[:, t, :], rhs=se1w[:, t, :],
                             start=(t == 0), stop=False)
        nc.tensor.matmul(seh_p[:], lhsT=ones1[:], rhs=brw(OFF_SE1B, RED),
                         start=False, stop=True)
        seh = tiny.tile([B, RED], f32, tag="seh")
        nc.scalar.activation(out=seh[:], in_=seh_p[:], func=AF.Relu)
        pt = ppt.tile([128, B], f32, tag="pt")
        nc.tensor.transpose(pt[:RED, :], seh[:], ident[:])
        sehT = tiny.tile([RED, B], bf16, tag="sehT")
        nc.vector.tensor_copy(out=sehT[:], in_=pt[:RED, :])
        se_p = pm.tile([B, C], f32, tag="pm1k")
        for n in range(2):
            nc.tensor.matmul(se_p[:, n * 512:(n + 1) * 512], lhsT=sehT[:],
                             rhs=se2w[:, n, :], start=True, stop=False)
            nc.tensor.matmul(se_p[:, n * 512:(n + 1) * 512], lhsT=ones1[:],
                             rhs=brw(OFF_SE2B + n * 512, 512), start=False, stop=True)
        se_t = singles.tile([B, C], bf16, tag="se")

        # in_proj (conv center-tap folded into xs columns; conv_b as K=1 row)
        u_pre = singles.tile([B, C], bf16, tag="upre")
        z_pre = singles.tile([B, C], bf16, tag="zpre")
        for g in range(4):
            xz_p = pm5.tile([B, 2 * DG], f32, tag="pm512")
            for t in range(2):
                gt = 2 * g + t
                nc.tensor.matmul(xz_p[:], lhsT=xnT[:, gt, :], rhs=ipw[:, gt, :],
                                 start=(t == 0), stop=False)
            nc.tensor.matmul(xz_p[:], lhsT=ones1[:], rhs=brw(OFF_CB + g * 512, 512),
                             start=False, stop=True)
            sl = slice(g * DG, (g + 1) * DG)
            nc.vector.tensor_copy(out=u_pre[:, sl], in_=xz_p[:, :DG])
            nc.vector.tensor_copy(out=z_pre[:, sl], in_=xz_p[:, DG:])

        # u = silu(u_pre)
        u_all = singles.tile([B, C], bf16, tag="uall")
        sigmoid_into(u_all, u_pre[:], C)
        nc.vector.tensor_mul(out=u_all[:], in0=u_all[:], in1=u_pre[:])
        uT = transpose_in(u_all[:], C, tag="uT", in_bf16=True)

        # off-critical-path work emitted here (PE is busy with x_dbl/dt):
        # the SE sigmoid and the full xn tensor for the y2 multiply
        sigmoid_into(se_t, se_p[:], C)
        tap(1, se_t[:])
        xn = singles.tile([B, C], bf16, tag="xn")
        nc.vector.tensor_mul(out=xn[:], in0=xnr[:], in1=vrow(R_GMW))
        nc.vector.tensor_add(out=xn[:], in0=xn[:], in1=vrow(R_GMB))
        tap(0, xn[:])

        # x_dbl: one [16,4,18] psum; dts gathered into [16,65] with ones col
        dtscat = singles.tile([B, 4 * DTRANK + 1], f32, tag="dtscat")
        nc.vector.memset(dtscat[:, 4 * DTRANK:], 1.0)
        xdb_p = pm5.tile([B, 4, DTRANK + 2], f32, tag="pm512")
        for g in range(4):
            for t in range(2):
                nc.tensor.matmul(xdb_p[:, g, :], lhsT=uT[:, 2 * g + t, :],
                                 rhs=xpw[:, 2 * g + t, :],
                                 start=(t == 0), stop=(t == 1))
        bcx = tiny.tile([B, 4, 2], f32, tag="bcx")
        nc.vector.tensor_copy(out=bcx[:], in_=xdb_p[:, :, DTRANK:DTRANK + 2])
        bc4 = tiny.tile([B, 4], f32, tag="bc4")
        nc.vector.tensor_mul(out=bc4[:], in0=bcx[:, :, 0:1].rearrange("b g o -> b (g o)"),
                             in1=bcx[:, :, 1:2].rearrange("b g o -> b (g o)"))
        for g in range(4):
            nc.vector.tensor_copy(out=dtscat[:, g * DTRANK:(g + 1) * DTRANK],
                                  in_=xdb_p[:, g, :DTRANK])
        ptd = ppt.tile([128, B], f32, tag="pt")
        nc.tensor.transpose(ptd[:4 * DTRANK + 1, :], dtscat[:], ident[:])
        dtsT = tiny.tile([4 * DTRANK + 1, B], bf16, tag="dtsT")
        nc.vector.tensor_copy(out=dtsT[:], in_=ptd[:4 * DTRANK + 1, :])

        # delta_in = dts@blockdiag(dtw) + dtb  (ones row); then
        # y = u * (softplus(delta_in) * B*C + D)
        dl_p = pm.tile([B, C], f32, tag="pm1k")
        for n in range(2):
            nc.tensor.matmul(dl_p[:, n * 512:(n + 1) * 512], lhsT=dtsT[:],
                             rhs=dtwa[:, n * 512:(n + 1) * 512], start=True, stop=True)
        y_t = singles.tile([B, C], bf16, tag="y")
        for h in range(2):
            sl = slice(h * 512, (h + 1) * 512)
            nc.scalar.activation(out=y_t[:, sl], in_=dl_p[:, sl], func=AF.Exp)
            nc.scalar.activation(out=y_t[:, sl], in_=y_t[:, sl], func=AF.Ln,
                                 bias=smal_t[:, 4:5], scale=1.0)
        for g in range(4):
            sl = slice(g * DG, (g + 1) * DG)
            nc.vector.scalar_tensor_tensor(
                out=y_t[:, sl], in0=y_t[:, sl], scalar=bc4[:, g:g + 1],
                in1=vrow(R_D, DG, g * DG), op0=ALU.mult, op1=ALU.add)
        nc.vector.tensor_mul(out=y_t[:], in0=y_t[:], in1=u_all[:])
        tap(2, y_t[:])

        # sz = silu(z_pre)  (emitted late: DVE/ACT free while PE does x_dbl)
        sz = singles.tile([B, C], bf16, tag="sz")
        sigmoid_into(sz, z_pre[:], C)
        nc.vector.tensor_mul(out=sz[:], in0=sz[:], in1=z_pre[:])

        # per-group out-norm LN (stats batched across the 4 groups), * silu(z)
        yn = a1k.tile([B, C], bf16, tag="a1kb")
        mv4 = stats.tile([B, 4, 2], f32, tag="mv4")
        for g in range(4):
            st_g = stats.tile([B, 6], f32, tag="st6")
            nc.vector.bn_stats(out=st_g[:], in_=y_t[:, g * DG:(g + 1) * DG])
            nc.vector.bn_aggr(out=mv4[:, g, :], in_=st_g[:])
        nc.scalar.activation(out=mv4[:, :, 1:2], in_=mv4[:, :, 1:2], func=AF.Ln,
                             bias=smal_t[:, 3:4], scale=1.0)
        nc.scalar.activation(out=mv4[:, :, 1:2], in_=mv4[:, :, 1:2], func=AF.Exp,
                             scale=-0.5)
        nm4 = stats.tile([B, 4], f32, tag="nm4")
        nc.vector.scalar_tensor_tensor(
            out=nm4[:], in0=mv4[:, :, 0:1].rearrange("b g o -> b (g o)"),
            scalar=-1.0, in1=mv4[:, :, 1:2].rearrange("b g o -> b (g o)"),
            op0=ALU.mult, op1=ALU.mult)
        for g in range(4):
            sl = slice(g * DG, (g + 1) * DG)
            nc.scalar.activation(out=yn[:, sl], in_=y_t[:, sl], func=AF.Identity,
                                 bias=nm4[:, g:g + 1], scale=mv4[:, g, 1:2])
        nc.vector.tensor_mul(out=yn[:], in0=yn[:], in1=vrow(R_ONW))
        nc.vector.tensor_add(out=yn[:], in0=yn[:], in1=vrow(R_ONB))
        nc.vector.tensor_mul(out=yn[:], in0=yn[:], in1=sz[:])

        # out_proj per group
        yzT = transpose_in(yn[:], C, tag="yzT", in_bf16=True)
        ycat = a1k.tile([B, C], bf16, tag="a1kb")
        for g in range(4):
            ys_p = pm5.tile([B, DG], f32, tag="pm512")
            for t in range(2):
                nc.tensor.matmul(ys_p[:], lhsT=yzT[:, 2 * g + t, :],
                                 rhs=opw[:, 2 * g + t, :],
                                 start=(t == 0), stop=(t == 1))
            nc.vector.tensor_copy(out=ycat[:, g * DG:(g + 1) * DG], in_=ys_p[:])

        # y2 = ycat * skip * xn * se;  y3 = LN-raw(y2)  (gain/bias folded
        # into gm weights host-side)
        nc.vector.scalar_tensor_tensor(
            out=ycat[:], in0=ycat[:], scalar=smal_t[:, 2:3], in1=xn[:],
            op0=ALU.mult, op1=ALU.mult)
        nc.vector.tensor_mul(out=ycat[:], in0=ycat[:], in1=se_t[:])
        y3 = a1k.tile([B, C], bf16, tag="a1kb")
        nm3, mv3 = ln_stats(ycat[:], C)
        ln_apply(ycat[:], y3[:], nm3, mv3)

        # a = y3raw @ gm'  (+ bias row)
        y3T = transpose_in(y3[:], C, tag="y3T", in_bf16=True)
        a_p = pm.tile([B, C], f32, tag="pm1k")
        for n in range(2):
            for t in range(8):
                nc.tensor.matmul(a_p[:, n * 512:(n + 1) * 512], lhsT=y3T[:, t, :],
                                 rhs=gmw[:, t, n * 512:(n + 1) * 512],
                                 start=(t == 0), stop=False)
            nc.tensor.matmul(a_p[:, n * 512:(n + 1) * 512], lhsT=ones1[:],
                             rhs=brw(OFF_GMB + n * 512, 512), start=False, stop=True)

        # cls1 = (cls + n1b) + LN(a)*n1w   (cls+norm1_b precomputed on host)
        aln = a1k.tile([B, C], bf16, tag="a1kb")
        nma, mva = ln_stats(a_p[:], C)
        ln_apply(a_p[:], aln[:], nma, mva)
        nc.vector.tensor_mul(out=aln[:], in0=aln[:], in1=vrow(R_N1W))
        cls1 = singles.tile([B, C], bf16, tag="cls1")
        nc.vector.tensor_add(out=cls1[:], in0=clsb_t[:], in1=aln[:])
        tap(4, cls1[:])

        # select this core's rows of cls1 and ship them; the reduced MLP
        # rows leave the device separately (h2o) and the 2-row residual add
        # happens on the host during gather.
        fin_p = pm.tile([BPC, C], f32, tag="pm1k")
        for n in range(2):
            sl = slice(n * 512, (n + 1) * 512)
            nc.tensor.matmul(fin_p[:, sl], lhsT=selb_t[:], rhs=cls1[:, sl],
                             start=True, stop=True)
        orow = tiny.tile([BPC, C], f32, tag="orow")
        nc.scalar.copy(out=orow[:], in_=fin_p[:])
        nc.scalar.dma_start(out=out_h[:, :], in_=orow[:])

        # h = LN-raw(cls1)  (norm2 gain/bias folded into fc1 host-side)
        h_t = a1k.tile([B, C], bf16, tag="a1kb")
        nmh, mvh = ln_stats(cls1[:], C)
        ln_apply(cls1[:], h_t[:], nmh, mvh)
        hT = transpose_in(h_t[:], C, tag="hT", in_bf16=True)

        # fc1 shard + gelu(sigmoid approx)
        h1_p = pm5.tile([B, FC1_SH], f32, tag="pm512")
        for t in range(8):
            nc.tensor.matmul(h1_p[:], lhsT=hT[:, t, :], rhs=fc1[:, t, :],
                             start=(t == 0), stop=False)
        nc.tensor.matmul(h1_p[:], lhsT=ones1[:], rhs=brw(OFF_FC1B, FC1_SH),
                         start=False, stop=True)
        h1s = tiny.tile([B, FC1_SH], bf16, tag="h1s")
        sigmoid_into(h1s, h1_p[:], FC1_SH, scale=1.702)
        h1 = tiny.tile([B, FC1_SH], bf16, tag="h1")
        nc.vector.tensor_mul(out=h1[:], in0=h1s[:], in1=h1_p[:])
        tap(5, h1[:], FC1_SH)

        # fc2 shard partial (+ fc2_b/8 so the ReduceScatter applies the bias)
        h1T = transpose_in(h1[:], FC1_SH, tag="h1T", in_bf16=True)
        p_p = pm.tile([B, C], f32, tag="pm1k")
        for n in range(2):
            for t in range(4):
                nc.tensor.matmul(p_p[:, n * 512:(n + 1) * 512], lhsT=h1T[:, t, :],
                                 rhs=fc2[:, t, n * 512:(n + 1) * 512],
                                 start=(t == 0), stop=False)
            nc.tensor.matmul(p_p[:, n * 512:(n + 1) * 512], lhsT=ones1[:],
                             rhs=brw(OFF_FC2B + n * 512, 512), start=False, stop=True)
        p_s = a1k.tile([B, C], bf16, tag="a1kb")
        cc_in = dram.tile([B, C], bf16, tag="cc_in")
        # halves pipelined: copy h0, DMA h0 (idle SP ring) while copying h1
        nc.scalar.copy(out=p_s[:, :512], in_=p_p[:, :512])
        nc.sync.dma_start(out=cc_in[:, :512], in_=p_s[:, :512])
        nc.scalar.copy(out=p_s[:, 512:], in_=p_p[:, 512:])
        nc.sync.dma_start(out=cc_in[:, 512:], in_=p_s[:, 512:])
        cc_out = dram.tile([BPC, C], bf16, tag="cc_out")
        nc.gpsimd.collective_compute(
            "ReduceScatter", mybir.AluOpType.add,
            replica_groups=[list(range(NCORES))],
            ins=[cc_in[:].opt()], outs=[cc_out[:].opt()],
        )
        nc.sync.dma_start(out=h2o_h[:], in_=cc_out[:])

    nc.compile()
    return nc


def _prepare_in_maps(inputs):
    import ml_dtypes

    def _w(a):
        return np.ascontiguousarray(_f32(a).astype(ml_dtypes.bfloat16))

    x = np.asarray(inputs["x"])
    cls_all = _f32(x[:, 0, :])
    cw_center = _f32(inputs["ss_conv_w"])[:, :, 1, 1]        # [4, 256]
    conv_b = _f32(inputs["ss_conv_b"])                        # [4, 256]
    gmw_n = _f32(inputs["gm_norm_w"])
    gmb_n = _f32(inputs["gm_norm_b"])
    n2w = _f32(inputs["norm2_w"])
    n2b = _f32(inputs["norm2_b"])
    gm_proj_w = _f32(inputs["gm_proj_w"])
    dt_w = _f32(inputs["ss_dt_w"])                            # [4, 16, 256]
    dt_b = _f32(inputs["ss_dt_b"])                            # [4, 256]
    fc1_w = _f32(inputs["mlp_fc1_w"])
    fc1_b = _f32(inputs["mlp_fc1_b"])
    fc2_w = _f32(inputs["mlp_fc2_w"])
    fc2_b = _f32(inputs["mlp_fc2_b"])

    # conv center tap folded into the xs half of in_proj columns, then
    # gm_norm gain folded into the rows (the matmul consumes raw-LN xnr);
    # gm_norm bias lands in the conv-bias row.
    ipw_host = _f32(inputs["ss_in_proj"]).copy()              # [4, 256, 512]
    ip_bias = np.zeros((4, 2 * DG), np.float32)
    for g in range(4):
        ipw_host[g][:, :DG] *= cw_center[g][None, :]
        gsl = slice(g * DG, (g + 1) * DG)
        ip_bias[g] = gmb_n[gsl] @ ipw_host[g]
        ipw_host[g] *= gmw_n[gsl][:, None]

    # gm_norm folded into the SE first layer likewise
    se1w_host = _f32(inputs["se_fc1_w"]) * gmw_n[:, None]
    se1b_host = gmb_n @ _f32(inputs["se_fc1_w"]) + _f32(inputs["se_fc1_b"])

    # dt blockdiag + dtb ones-row
    dtwa = np.zeros((4 * DTRANK + 1, C), np.float32)
    for g in range(4):
        dtwa[g * DTRANK:(g + 1) * DTRANK, g * DG:(g + 1) * DG] = dt_w[g]
    dtwa[4 * DTRANK, :] = dt_b.reshape(-1)

    # y3-LN gain folded into gm_proj rows; bias -> row vector
    gmw_host = gm_proj_w * gmw_n[:, None]
    gm_bias = gmb_n @ gm_proj_w + _f32(inputs["gm_proj_b"])

    # norm2 gain folded into fc1 rows
    fc1_host = fc1_w * n2w[:, None]

    vecs = np.zeros((NV, 1024), np.float32)
    vecs[R_GMW] = gmw_n
    vecs[R_GMB] = gmb_n
    vecs[R_N1W] = _f32(inputs["norm1_w"])
    vecs[R_D] = _f32(inputs["ss_D"]).reshape(-1)
    vecs[R_ONW] = _f32(inputs["ss_out_norm_w"]).reshape(-1)
    vecs[R_ONB] = _f32(inputs["ss_out_norm_b"]).reshape(-1)

    brow_base = np.zeros((NBROW,), np.float32)
    for g in range(4):
        brow_base[OFF_CB + g * 512: OFF_CB + g * 512 + 2 * DG] = ip_bias[g]
        brow_base[OFF_CB + g * 512: OFF_CB + g * 512 + DG] += conv_b[g]
    brow_base[OFF_SE1B:OFF_SE1B + RED] = se1b_host
    brow_base[OFF_SE2B:OFF_SE2B + C] = _f32(inputs["se_fc2_b"])
    brow_base[OFF_GMB:OFF_GMB + C] = gm_bias
    brow_base[OFF_FC2B:OFF_FC2B + C] = fc2_b / NCORES

    skip = float(_f32(inputs["skip_scale"]).reshape(-1)[0])

    shared = {
        "cls_all": cls_all,
        "clsb": _f32(cls_all + _f32(inputs["norm1_b"])[None, :]),
        "ident16": np.eye(B, dtype=np.float32),
        "vecs": np.ascontiguousarray(_w(vecs).reshape(-1)),
        "se1w": _w(se1w_host),
        "se2w": _w(inputs["se_fc2_w"]),
        "ipw": _w(ipw_host),
        "xpw": _w(inputs["ss_x_proj"]),
        "dtwa": _w(dtwa),
        "opw": _w(inputs["ss_out_proj"]),
        "gmw": _w(gmw_host),
    }

    in_maps = []
    for i in range(NCORES):
        sh = slice(i * FC1_SH, (i + 1) * FC1_SH)
        brow = brow_base.copy()
        brow[OFF_FC1B:OFF_FC1B + FC1_SH] = n2b @ fc1_w[:, sh] + fc1_b[sh]
        smal = np.zeros((B, 6), np.float32)
        for j in range(BPC):
            smal[i * BPC + j, j] = 1.0
        smal[:, 2] = skip
        smal[:, 3] = EPS
        smal[:, 4] = 1.0
        m = dict(shared)
        m.update({
            "smal": smal,
            "selb": _w(smal[:, 0:2]),
            "mod2b": _w(np.eye(2, dtype=np.float32)[np.arange(B) % 2]),
            "brow": np.ascontiguousarray(_w(brow).reshape(1, NBROW)),
            "fc1s": _w(fc1_host[:, sh]),
            "fc2s": _w(fc2_w[i * FC2_SH:(i + 1) * FC2_SH, :]),
        })
        in_maps.append(m)
    return in_maps


def _install_trace_shims():
    """This image lacks ``antenv.axon_hooks`` and fish-bucket access; stub in
    the ctypes NTFF hook from trn_boot and make artifact upload a no-op."""
    import sys
    import types

    import concourse.bass_utils as bu

    bu.upload_artifacts = lambda tmpdir: f"local:{tmpdir}"
    if "antenv.axon_hooks" not in sys.modules:
        from trn_agent_boot.trn_boot import _ntff_profile_via_ctypes

        mod = types.ModuleType("antenv.axon_hooks")
        hook = _ntff_profile_via_ctypes("/opt/axon/libaxon_pjrt.so")
        mod.get_axon_ntff_profile_hook = lambda: hook
        mod.set_axon_ntff_profile_hook = lambda h: None
        sys.modules["antenv.axon_hooks"] = mod
        import antenv

        antenv.axon_hooks = mod


def kernel(**inputs):
    global LAST_RESULT
    from concourse.bass_utils import run_bass_kernel_spmd

    key = "dbg" if DEBUG_TAPS else "plain"
    if key not in _CACHE:
        _CACHE[key] = _build(DEBUG_TAPS)
    nc = _CACHE[key]

    kwargs = {}
    if TRACE:
        _install_trace_shims()
        tdir = "/root/problem/.trace_" + key
        import os
        import shutil

        shutil.rmtree(tdir, ignore_errors=True)
        os.makedirs(tdir, exist_ok=True)
        kwargs = {"tmpdir": tdir}

    in_maps = _prepare_in_maps(inputs)
    res = run_bass_kernel_spmd(nc, in_maps, list(range(NCORES)), trace=TRACE, **kwargs)
    LAST_RESULT = res
    # device computed only the cls rows; the tail is the identity.
    # Each core ships its selected cls1 rows and its ReduceScatter'd MLP
    # rows separately; the final 2-row residual add happens here.
    out = np.array(inputs["x"], dtype=np.float32, copy=True)
    out[:, 0, :] = np.concatenate(
        [res.results[i]["out"] + np.asarray(res.results[i]["h2o"], dtype=np.float32)
         for i in range(NCORES)], axis=0)
    return out
